# revision 63
# baseline (speedup 1.0000x reference)
"""Trainium2 Bass kernel for nn_Attention_cross (dual-branch cross-reuse attention).

Reference computation (B=4, N0=2048, C=768, H=12, hd=64, N=1024):
  x_diff, x_cond = x[:, :N], x[:, N:]
  q,k,v per branch = x @ w_qkv (per-head), attn = softmax(q k^T / sqrt(hd))
  o_d = ((attn_diff @ v_d) @ w_proj_diff + b_d) reused per-head with attn_cond
  o_c = (attn_cond @ v_c) @ w_proj_cond + b_c
  out = concat([o_d, o_c], axis=1)

Sharding: 8 cores = 4 batches x 2 head-groups (6 heads each). The
head-mixing projections are row-sharded with bf16 pair ReduceScatters
(one per branch-half) whose serial 4x(15us+bytes) chain dominates the
schedule: the kernel is ordered [QKV_d, attn_d (with the cond QKV
tasks interleaved one-per-unit into the exp-bound units' PE slack),
attn_c nb0, attn_c nb1, o2-drain] so the first reduce is emitted as
early as possible and the four reduces run back to back behind
compute.  x arrives pre-transposed from the host so QKV starts
immediately with no on-chip transposes; QKV matmuls accumulate
straight into the scores PSUM ring.  Attention runs on exp(scores)
(softmax division deferred): the AV product is computed TRANSPOSED
(eT stationary, v moving with a ones column) so row sums land
per-partition and normalization is one strided reciprocal + one
stride-0-broadcast DVE multiply; PE transposes stage the result into
the d-major layout the projection consumes.  The second attention
(o2) is transposed the same way and written straight to the natural
o_d layout while the last ReduceScatter runs.  Projection partials
carry bias/2 each so the pair reduce completes the bias; p_d is
consumed directly from the reduce pins.  All matmul inputs bf16 (f32
PSUM accumulation); exp batched 2 PSUM banks per ACT instruction;
psum->SBUF copies and proj stores run on DVE keeping ACT pure-exp.
"""
import numpy as np

import concourse.bass as bass
import concourse.tile as tile
from concourse import bacc, mybir
from concourse.bass_utils import run_bass_kernel_spmd
from concourse.masks import make_identity
from concourse.alu_op_type import AluOpType

F32 = mybir.dt.float32
BF16 = mybir.dt.bfloat16
Exp = mybir.ActivationFunctionType.Exp
Copy = mybir.ActivationFunctionType.Copy

B, N0, C = 4, 2048, 768
H, HD = 12, 64
N = N0 // 2              # 1024 sequence per branch
HPC = H // 2             # 6 heads per core
CW = HPC * HD            # 384 own C-columns/rows
NCH = N // 128           # 8 chunks of 128 along n/m
CCH = C // 128           # 6 chunks of 128 along C
NB = N // 512            # 2 blocks of 512 along n
NH = N // 2              # 512 rows per RS chunk
SCALE = HD ** -0.5

N_CORES = 8
GROUPS = [[0, 1], [2, 3], [4, 5], [6, 7]]

_CACHE = {}


def _build():
    nc = bacc.Bacc("TRN2", target_bir_lowering=False, debug=False,
                   num_devices=N_CORES)

    # x arrives TRANSPOSED from host: [128, CCH, N0] (c-chunk-major cols)
    xt = nc.dram_tensor("xt", [128, CCH, N0], BF16, kind="ExternalInput").ap()
    wqk_d = nc.dram_tensor("wqk_d", [128, CCH, 2 * CW], BF16, kind="ExternalInput").ap()
    wqk_c = nc.dram_tensor("wqk_c", [128, CCH, 2 * CW], BF16, kind="ExternalInput").ap()
    wv_d = nc.dram_tensor("wv_d", [128, CCH, CW], BF16, kind="ExternalInput").ap()
    wv_c = nc.dram_tensor("wv_c", [128, CCH, CW], BF16, kind="ExternalInput").ap()
    wp_d = nc.dram_tensor("wp_d", [128, CW // 128, C], BF16, kind="ExternalInput").ap()
    wp_c = nc.dram_tensor("wp_c", [128, CW // 128, C], BF16, kind="ExternalInput").ap()
    bias_d = nc.dram_tensor("bias_d", [1, CW], BF16, kind="ExternalInput").ap()
    bias_c = nc.dram_tensor("bias_c", [1, CW], BF16, kind="ExternalInput").ap()
    # o_d in natural [N, CW] layout (the transposed second attention
    # produces n-major chunks directly)
    o_d_nat = nc.dram_tensor("o_d_nat", [N, CW], BF16, kind="ExternalOutput").ap()
    o_c_full = nc.dram_tensor("o_c_cols", [N, CW], BF16, kind="ExternalOutput").ap()

    # ReduceScatter halves (bf16): input [2, NH, CW] slot-major; each pair
    # member receives its slot reduced: out [NH, CW].
    cc_in_d = [nc.dram_tensor(f"cc_in_d{k}", [2, NH, CW], BF16).ap() for k in range(2)]
    cc_out_d = [nc.dram_tensor(f"cc_out_d{k}", [NH, CW], BF16).ap() for k in range(2)]
    cc_in_c = [nc.dram_tensor(f"cc_in_c{k}", [2, NH, CW], BF16).ap() for k in range(2)]
    cc_out_c = [nc.dram_tensor(f"cc_out_c{k}", [NH, CW], BF16).ap() for k in range(2)]

    with tile.TileContext(nc) as tc:
        _body(nc, tc, xt, wqk_d, wqk_c, wv_d, wv_c, wp_d, wp_c,
              bias_d, bias_c, o_d_nat, o_c_full,
              cc_in_d, cc_out_d, cc_in_c, cc_out_c)
    nc.compile()
    return nc


def _body(nc, tc, xt, wqk_d, wqk_c, wv_d, wv_c, wp_d, wp_c,
          bias_d, bias_c, o_d_nat, o_c_full,
          cc_in_d, cc_out_d, cc_in_c, cc_out_c):
    from contextlib import ExitStack
    ctx = ExitStack()
    with ctx:
        ctx.enter_context(nc.allow_low_precision(reason="bf16 matmul inputs by design"))
        singles = ctx.enter_context(tc.tile_pool(name="singles", bufs=1))
        ident = singles.tile([128, 128], F32)
        make_identity(nc, ident[:])
        identb = singles.tile([128, 128], BF16)
        nc.vector.tensor_copy(identb[:], ident[:])

        big = ctx.enter_context(tc.tile_pool(name="big", bufs=1))
        qkT = {}    # branch -> [128, 6, N] bf16  rows: [q h0..h5 | k h0..h5]
        v_aug = {}  # branch -> [128, HPC, NCH, 65] bf16 (ones col -> row sums)
        u_t = {}    # branch -> [128, 3, N] bf16 (UNNORMALIZED (expS@v)^T)
        rT = {}     # recip row sums at partition 0, free-indexed by unit slot
        for br in ("d", "c"):
            v_aug[br] = big.tile([128, HPC, NCH, HD + 1], BF16, tag=f"v_{br}", name=f"v_{br}")
            nc.vector.memset(v_aug[br][:, :, :, HD:HD + 1], 1.0)
        qkT["c"] = big.tile([128, 2 * CW // 128, N], BF16, tag="qkT_c", name="qkT_c")
        u_t["c"] = big.tile([128, CW // 128, N], BF16, tag="u_c", name="u_c")
        u_t["d"] = big.tile([128, CW // 128, N], BF16, tag="u_d", name="u_d")
        # normalized AV output in n-major layout, staging for the transposes
        u_n = {}
        u_n["c"] = big.tile([128, NCH, HPC, HD], BF16, tag="un_c", name="un_c")
        u_n["d"] = big.tile([128, NCH, HPC, HD], BF16, tag="un_d", name="un_d")
        # per-partition reciprocal row sums [n-part, unit, chunk]; cond
        # persists into the o2 drain, diff is a small ring
        rT["c"] = big.tile([128, HPC * NB, 4], F32, tag="r_c", name="r_c")
        rT["d"] = big.tile([128, 4, 4], F32, tag="r_d", name="r_d")

        def rslot(br, h, nb):
            u = h * NB + nb
            return u % 4 if br == "d" else u

        wp_r = {}
        for br in ("d", "c"):
            wp_r[br] = big.tile([128, CW // 128, C], BF16, tag=f"wp_{br}", name=f"wp_{br}")
        bias_b = {}
        bias_b["d"] = big.tile([128, CW], BF16, tag="bias_bd", name="bias_bd")
        bias_b["c"] = big.tile([128, CW], BF16, tag="bias_bc", name="bias_bc")

        # ---------------- pools ----------------
        # qkT_d dies after the d-nb1 scores; the cond qkv load tiles die
        # after the mid-stream QKV_c; the cond eT ring reuses that region.
        dside = ctx.enter_context(tc.tile_pool(name="dside", bufs=1))
        qkT["d"] = dside.tile([128, 2 * CW // 128, N], BF16, tag="qkT_d", name="qkT_d")

        def eTd_pool_tile(tag, bufs, name):
            return dside.tile([128, NCH, 512], BF16, tag=tag, bufs=bufs, name=name)

        pj = ctx.enter_context(tc.tile_pool(name="pj", bufs=1))
        pdl = ctx.enter_context(tc.tile_pool(name="pdl", bufs=1))
        o2p = ctx.enter_context(tc.tile_pool(name="o2pool", bufs=2))
        ps_sc = ctx.enter_context(tc.tile_pool(name="ps_sc", bufs=2, space="PSUM"))
        ps_av = ctx.enter_context(tc.tile_pool(name="ps_av", bufs=2, space="PSUM"))
        ps_pj = ctx.enter_context(tc.tile_pool(name="ps_pj", bufs=2, space="PSUM"))

        # ---------------- emit helpers ----------------
        def load_qkv(br, wqk_r, wv_r, xT):
            wqk, wv = (wqk_d, wv_d) if br == "d" else (wqk_c, wv_c)
            half = 0 if br == "d" else N
            nc.sync.dma_start(out=wqk_r[:, :, 0:128], in_=wqk[:, :, 0:128])
            nc.sync.dma_start(out=xT[:, :, 0:512], in_=xt[:, :, half:half + 512])
            for f0 in range(128, 2 * CW, 320):
                f1 = min(f0 + 320, 2 * CW)
                nc.sync.dma_start(out=wqk_r[:, :, f0:f1], in_=wqk[:, :, f0:f1])
            nc.sync.dma_start(out=xT[:, :, 512:N], in_=xt[:, :, half + 512:half + N])
            nc.sync.dma_start(out=wv_r[:], in_=wv)

        def qkv_tasks(br, wqk_r, wv_r, xT):
            # QK: two fi-column groups per [128, 1024] scores-ring tile;
            # V: two m-chunks per tile.  Returns one closure per ring tile.
            def qk_pair(nb, fi):
                def emit():
                    ps = ps_sc.tile([128, 1024], F32, tag="sc_ps")
                    for k in range(2):
                        for ci in range(CCH):
                            nc.tensor.matmul(
                                ps[:, k * 512:(k + 1) * 512],
                                wqk_r[:, ci, (fi + k) * 128:(fi + k + 1) * 128],
                                xT[:, ci, nb * 512:(nb + 1) * 512],
                                start=(ci == 0), stop=(ci == CCH - 1))
                    nc.vector.tensor_copy(
                        qkT[br][:, fi:fi + 2, nb * 512:(nb + 1) * 512],
                        ps[:].rearrange("p (f q) -> p f q", f=2))
                return emit

            def v_pair(mch):
                def emit():
                    ps = ps_sc.tile([128, 1024], F32, tag="sc_ps")
                    for k in range(2):
                        for ci in range(CCH):
                            nc.tensor.matmul(
                                ps[:, k * 512:k * 512 + CW],
                                xT[:, ci, (mch + k) * 128:(mch + k + 1) * 128],
                                wv_r[:, ci, :],
                                start=(ci == 0), stop=(ci == CCH - 1))
                    for k in range(2):
                        nc.vector.tensor_copy(
                            v_aug[br][:, :, mch + k, 0:HD],
                            ps[:, k * 512:k * 512 + CW]
                            .rearrange("p (h d) -> p h d", h=HPC))
                return emit

            def qk_single(nb, fi):
                def emit():
                    ps = ps_sc.tile([128, 1024], F32, tag="sc_ps")
                    for ci in range(CCH):
                        nc.tensor.matmul(
                            ps[:, 0:512],
                            wqk_r[:, ci, fi * 128:(fi + 1) * 128],
                            xT[:, ci, nb * 512:(nb + 1) * 512],
                            start=(ci == 0), stop=(ci == CCH - 1))
                    nc.vector.tensor_copy(
                        qkT[br][:, fi, nb * 512:(nb + 1) * 512], ps[:, 0:512])
                return emit

            tasks = [qk_single(0, 0)]
            for nb in range(NB):
                start_fi = 1 if nb == 0 else 0
                for fi in range(start_fi, 2 * CW // 128 - 1, 2):
                    tasks.append(qk_pair(nb, fi))
                if (2 * CW // 128 - start_fi) % 2 == 1:
                    tasks.append(qk_single(nb, 2 * CW // 128 - 1))
            for mch in range(0, NCH, 2):
                tasks.append(v_pair(mch))
            return tasks

        def emit_scores(br, h, nb, eT_pool, tag, bufs):
            qc, qo = divmod(h * HD, 128)
            kc, ko = divmod(CW + h * HD, 128)
            eT = eT_pool.tile([128, NCH, 512], BF16, tag=tag, bufs=bufs,
                              name=f"eT_{br}_{h}_{nb}")
            for pr in range(NCH // 2):
                ps = ps_sc.tile([128, 1024], F32, tag="sc_ps")
                for k in range(2):
                    mch = pr * 2 + k
                    nc.tensor.matmul(
                        ps[:, k * 512:(k + 1) * 512],
                        qkT[br][ko:ko + HD, kc, mch * 128:(mch + 1) * 128],
                        qkT[br][qo:qo + HD, qc, nb * 512:(nb + 1) * 512],
                        start=True, stop=True)
                nc.scalar.activation(
                    eT[:, pr * 2:pr * 2 + 2, :].rearrange("p m q -> p (m q)"),
                    ps[:], Exp)
            return eT

        def emit_avmm(br, h, nb, eT):
            # TRANSPOSED AV: out[n, d] with eT as stationary, v as moving --
            # the moving width is only 65 (64 dims + ones column for the row
            # sums), so this runs at half the moving cost of the d-major
            # form.  Row sums land per-partition: normalization is a single
            # strided reciprocal + one broadcast multiply on DVE, no PE
            # expander needed.
            av = ps_av.tile([128, 4, HD + 1], F32, tag="av_ps")
            for j in range(4):
                for mch in range(NCH):
                    nc.tensor.matmul(
                        av[:, j, :],
                        eT[:, mch, j * 128:(j + 1) * 128],
                        v_aug[br][:, h, mch, :],
                        start=(mch == 0), stop=(mch == NCH - 1))
            slot = rslot(br, h, nb)
            nc.vector.reciprocal(rT[br][:, slot, :], av[:, :, HD])
            rb = rT[br][:, slot, :].rearrange("p (j o) -> p j o", o=1) \
                .broadcast_to([128, 4, HD])
            nc.vector.tensor_tensor(
                out=u_n[br][:, nb * 4:(nb + 1) * 4, h, :],
                in0=av[:, :, 0:HD], in1=rb, op=mybir.AluOpType.mult)
            return av

        def emit_fin(br, h, nb, av):
            # transpose the normalized n-major chunk into the d-major uT the
            # projection consumes (4 PE transposes + one DVE copy); the tp
            # tiles share the av psum ring (identical slot size)
            tp = ps_av.tile([HD, 4, 130], BF16, tag="av_ps")
            for j in range(4):
                nc.tensor.transpose(
                    tp[:, j, 0:128], u_n[br][:, nb * 4 + j, h, :], identb[:])
            uc, uo = divmod(h * HD, 128)
            nc.vector.tensor_copy(
                u_t[br][uo:uo + HD, uc, nb * 512:(nb + 1) * 512]
                .rearrange("p (j q) -> p j q", j=4),
                tp[:, :, 0:128])

        def proj_half(br, half):
            # bias/2 folded into each partial (the pair reduce completes it);
            # stores on DVE; one DMA per slot so the RS has few waits
            u = u_t[br]
            wp = wp_r[br]
            st = pj.tile([128, 2, 4, CW], BF16, tag="pj_st")
            for nch in range(half * 4, half * 4 + 4):
                j = nch - half * 4
                for slot in range(2):
                    ps = ps_pj.tile([128, CW], F32, tag="pj_ps")
                    last = CW // 128 - 1
                    for ci in range(CW // 128):
                        nc.tensor.matmul(
                            ps[:],
                            u[:, ci, nch * 128:(nch + 1) * 128],
                            wp[:, ci, slot * CW:(slot + 1) * CW],
                            start=(ci == 0), stop=(ci == last))
                    nc.vector.tensor_add(st[:, slot, j, :], ps[:], bias_b[br][:])
            cc = cc_in_d[half] if br == "d" else cc_in_c[half]
            for slot in range(2):
                nc.sync.dma_start(
                    out=cc[slot].rearrange("(j p) f -> p j f", p=128),
                    in_=st[:, slot, :, :])
            nc.gpsimd.collective_compute(
                "ReduceScatter", AluOpType.add, replica_groups=GROUPS,
                ins=[cc], outs=[(cc_out_d if br == "d" else cc_out_c)[half]])

        pins = {}

        def pd_pin(k):
            # gpsimd SWDGE queue: this DMA waits on the diff ReduceScatter
            # and must not head-of-line block the SP store queue
            pin = pdl.tile([128, 4, CW], BF16, tag="p_in", bufs=2,
                           name=f"pin_{k}")
            nc.gpsimd.dma_start(
                out=pin[:],
                in_=cc_out_d[k].rearrange("(j p) f -> p j f", p=128))
            pins[k] = pin

        def emit_o2(h, nb, eT):
            # TRANSPOSED o2 = (attn_cond @ p_d) per head: eT stationary,
            # p_d columns moving (64 wide); normalized by the per-partition
            # r_c and stored straight into the natural o_d layout
            ps_o = ps_av.tile([128, 4, HD + 1], F32, tag="av_ps")
            for j in range(4):
                for mch in range(NCH):
                    pin = pins[0] if mch < 4 else pins[1]
                    nc.tensor.matmul(
                        ps_o[:, j, 0:HD],
                        eT[:, mch, j * 128:(j + 1) * 128],
                        pin[:, mch % 4, h * HD:(h + 1) * HD],
                        start=(mch == 0), stop=(mch == NCH - 1))
            rb = rT["c"][:, h * NB + nb, :].rearrange("p (j o) -> p j o", o=1) \
                .broadcast_to([128, 4, HD])
            o2T = o2p.tile([128, 4, HD], BF16, tag="o2T")
            nc.vector.tensor_tensor(out=o2T[:], in0=ps_o[:, :, 0:HD], in1=rb,
                                    op=mybir.AluOpType.mult)
            nc.sync.dma_start(
                out=o_d_nat[nb * 512:(nb + 1) * 512, h * HD:(h + 1) * HD]
                .rearrange("(j p) f -> p j f", p=128),
                in_=o2T[:])

        # ---------------- the stream ----------------
        # task stream with a trailing avmm (lag 1) / fin (lag 2) discipline;
        # fin(h5, nb) triggers that half's projection + ReduceScatter
        avq = []
        finq = []
        hooks = {}

        def advance(scored=None):
            if avq:
                br, h, nb, eT = avq.pop(0)
                finq.append((br, h, nb, emit_avmm(br, h, nb, eT)))
            if len(finq) > 1 or (scored is None and finq):
                br, h, nb, ps_u = finq.pop(0)
                emit_fin(br, h, nb, ps_u)
                if h == HPC - 1:
                    proj_half(br, nb)
                    hk = hooks.pop((br, nb), None)
                    if hk:
                        hk()
            if scored is not None:
                avq.append(scored)

        with tc.tile_pool(name="cond_ld", bufs=1) as cond_ld, \
             tc.tile_pool(name="dld", bufs=1) as dld:
            wqk_cr = cond_ld.tile([128, CCH, 2 * CW], BF16, tag="wqk_c", name="wqk_cr")
            wv_cr = cond_ld.tile([128, CCH, CW], BF16, tag="wv_c", name="wv_cr")
            xT_c = cond_ld.tile([128, CCH, N], BF16, tag="xT_c", name="xT_c")
            wqk_dr = dld.tile([128, CCH, 2 * CW], BF16, tag="wqk_d", name="wqk_dr")
            wv_dr = dld.tile([128, CCH, CW], BF16, tag="wv_d", name="wv_dr")
            xT_d = dld.tile([128, CCH, N], BF16, tag="xT_d", name="xT_d")

            # ---- loads: diff first (its QKV starts the kernel), cond +
            # proj weights + bias stream behind on the same queue ----
            load_qkv("d", wqk_dr, wv_dr, xT_d)
            load_qkv("c", wqk_cr, wv_cr, xT_c)
            for br in ("d", "c"):
                nc.sync.dma_start(out=wp_r[br][:], in_=wp_d if br == "d" else wp_c)
            nc.sync.dma_start(out=bias_b["d"][:], in_=bias_d.to_broadcast([128, CW]))
            nc.sync.dma_start(out=bias_b["c"][:], in_=bias_c.to_broadcast([128, CW]))

            # ---- QKV_d, then the diff attention with the cond QKV tasks
            # interleaved one-per-unit into the exp-bound units' PE slack ----
            for t in qkv_tasks("d", wqk_dr, wv_dr, xT_d):
                t()

            ctasks = qkv_tasks("c", wqk_cr, wv_cr, xT_c)
            ci = 0
            hooks[("d", 1)] = lambda: (pd_pin(0), pd_pin(1))
            for nb in range(NB):
                for h in range(HPC):
                    eT = emit_scores("d", h, nb, dside, "eT_d", 2)
                    advance(("d", h, nb, eT))
                    if nb + h > 0 and ci < len(ctasks):
                        ctasks[ci]()
                        ci += 1
            while ci < len(ctasks):
                ctasks[ci]()
                ci += 1
                advance()

        # load pools closed: the cond eT ring reuses their region.
        # group order c-nb0, d-nb1, c-nb1 packs the four ReduceScatters
        # back to back (d0 early, then c0/d1/c1 as their fins complete)
        with tc.tile_pool(name="eTc", bufs=1) as eTc:
            o2q = []
            for nb in range(NB):
                for h in range(HPC):
                    eT = emit_scores("c", h, nb, eTc, "eT_c", 2 * HPC)
                    o2q.append((h, nb, eT))
                    advance(("c", h, nb, eT))
            while avq or finq:
                advance()

            # ---- drain: all 12 second-attention units overlap the
            # last cond ReduceScatter ----
            nc.gpsimd.dma_start(out=o_c_full[0:NH, :], in_=cc_out_c[0])
            for h, nb, eT in o2q:
                emit_o2(h, nb, eT)
            nc.sync.dma_start(out=o_c_full[NH:NH + 256, :],
                              in_=cc_out_c[1][0:256, :])
            nc.gpsimd.dma_start(out=o_c_full[NH + 256:N, :],
                                in_=cc_out_c[1][256:NH, :])


def _prep_inputs(x, w_qkv_diff, w_qkv_cond, w_proj_diff, b_proj_diff,
                 w_proj_cond, b_proj_cond):
    import ml_dtypes
    bf = ml_dtypes.bfloat16
    in_maps = []
    wqk = {}
    wvv = {}
    for hg in range(2):
        s = slice(hg * CW, (hg + 1) * CW)
        sk = slice(C + hg * CW, C + (hg + 1) * CW)
        sv = slice(2 * C + hg * CW, 2 * C + (hg + 1) * CW)
        for br, w in (("d", w_qkv_diff), ("c", w_qkv_cond)):
            wqk[(br, hg)] = np.ascontiguousarray(
                np.concatenate([w[:, s] * SCALE, w[:, sk]], axis=1)
                .reshape(CCH, 128, 2 * CW).transpose(1, 0, 2).astype(bf))
            wvv[(br, hg)] = np.ascontiguousarray(
                w[:, sv].reshape(CCH, 128, CW).transpose(1, 0, 2).astype(bf))
    wp = {}
    for br, w in (("d", w_proj_diff), ("c", w_proj_cond)):
        for hg in range(2):
            s = slice(hg * CW, (hg + 1) * CW)
            wp[(br, hg)] = np.ascontiguousarray(
                w[s, :].reshape(CW // 128, 128, C).transpose(1, 0, 2).astype(bf))
    for c in range(N_CORES):
        b, hg = divmod(c, 2)
        s = slice(hg * CW, (hg + 1) * CW)
        m = {
            "xt": np.ascontiguousarray(
                x[b].T.reshape(CCH, 128, N0).transpose(1, 0, 2).astype(bf)),
            "wqk_d": wqk[("d", hg)],
            "wqk_c": wqk[("c", hg)],
            "wv_d": wvv[("d", hg)],
            "wv_c": wvv[("c", hg)],
            "wp_d": wp[("d", hg)],
            "wp_c": wp[("c", hg)],
            "bias_d": np.ascontiguousarray((0.5 * b_proj_diff[None, s]).astype(bf)),
            "bias_c": np.ascontiguousarray((0.5 * b_proj_cond[None, s]).astype(bf)),
        }
        in_maps.append(m)
    return in_maps


def kernel(x, w_qkv_diff, w_qkv_cond, w_proj_diff, b_proj_diff,
           w_proj_cond, b_proj_cond):
    x = np.asarray(x)
    w_qkv_diff = np.asarray(w_qkv_diff)
    w_qkv_cond = np.asarray(w_qkv_cond)
    w_proj_diff = np.asarray(w_proj_diff)
    b_proj_diff = np.asarray(b_proj_diff)
    w_proj_cond = np.asarray(w_proj_cond)
    b_proj_cond = np.asarray(b_proj_cond)

    if "nc" not in _CACHE:
        _CACHE["nc"] = _build()
    nc = _CACHE["nc"]
    in_maps = _prep_inputs(x, w_qkv_diff, w_qkv_cond, w_proj_diff,
                           b_proj_diff, w_proj_cond, b_proj_cond)
    res = run_bass_kernel_spmd(nc, in_maps, list(range(N_CORES))).results

    o_d = np.empty((B, N, C), np.float32)
    o_c = np.empty((B, N, C), np.float32)
    for c in range(N_CORES):
        b, hg = divmod(c, 2)
        o_d[b][:, hg * CW:(hg + 1) * CW] = np.asarray(res[c]["o_d_nat"]).astype(np.float32)
        o_c[b][:, hg * CW:(hg + 1) * CW] = np.asarray(res[c]["o_c_cols"]).astype(np.float32)
    return np.concatenate([o_d, o_c], axis=1)


# revision 64
# speedup vs baseline: 1.0011x; 1.0011x over previous
"""Trainium2 Bass kernel for nn_Attention_cross (dual-branch cross-reuse attention).

Reference computation (B=4, N0=2048, C=768, H=12, hd=64, N=1024):
  x_diff, x_cond = x[:, :N], x[:, N:]
  q,k,v per branch = x @ w_qkv (per-head), attn = softmax(q k^T / sqrt(hd))
  o_d = ((attn_diff @ v_d) @ w_proj_diff + b_d) reused per-head with attn_cond
  o_c = (attn_cond @ v_c) @ w_proj_cond + b_c
  out = concat([o_d, o_c], axis=1)

Sharding: 8 cores = 4 batches x 2 head-groups (6 heads each). The
head-mixing projections are row-sharded with bf16 pair ReduceScatters
(one per branch-half) whose serial 4x(15us+bytes) chain dominates the
schedule: the kernel is ordered [QKV_d, attn_d (with the cond QKV
tasks interleaved one-per-unit into the exp-bound units' PE slack),
attn_c nb0, attn_c nb1, o2-drain] so the first reduce is emitted as
early as possible and the four reduces run back to back behind
compute.  x arrives pre-transposed from the host so QKV starts
immediately with no on-chip transposes; QKV matmuls accumulate
straight into the scores PSUM ring.  Attention runs on exp(scores)
(softmax division deferred): the AV product is computed TRANSPOSED
(eT stationary, v moving with a ones column) so row sums land
per-partition and normalization is one strided reciprocal + one
stride-0-broadcast DVE multiply; PE transposes stage the result into
the d-major layout the projection consumes.  The second attention
(o2) is transposed the same way and written straight to the natural
o_d layout while the last ReduceScatter runs.  Projection partials
carry bias/2 each so the pair reduce completes the bias; p_d is
consumed directly from the reduce pins.  All matmul inputs bf16 (f32
PSUM accumulation); exp batched 2 PSUM banks per ACT instruction;
psum->SBUF copies and proj stores run on DVE keeping ACT pure-exp.
"""
import numpy as np

import concourse.bass as bass
import concourse.tile as tile
from concourse import bacc, mybir
from concourse.bass_utils import run_bass_kernel_spmd
from concourse.masks import make_identity
from concourse.alu_op_type import AluOpType

F32 = mybir.dt.float32
BF16 = mybir.dt.bfloat16
Exp = mybir.ActivationFunctionType.Exp
Copy = mybir.ActivationFunctionType.Copy

B, N0, C = 4, 2048, 768
H, HD = 12, 64
N = N0 // 2              # 1024 sequence per branch
HPC = H // 2             # 6 heads per core
CW = HPC * HD            # 384 own C-columns/rows
NCH = N // 128           # 8 chunks of 128 along n/m
CCH = C // 128           # 6 chunks of 128 along C
NB = N // 512            # 2 blocks of 512 along n
NH = N // 2              # 512 rows per RS chunk
SCALE = HD ** -0.5

N_CORES = 8
GROUPS = [[0, 1], [2, 3], [4, 5], [6, 7]]

_CACHE = {}


def _build():
    nc = bacc.Bacc("TRN2", target_bir_lowering=False, debug=False,
                   num_devices=N_CORES)

    # x arrives TRANSPOSED from host: [128, CCH, N0] (c-chunk-major cols)
    xt = nc.dram_tensor("xt", [128, CCH, N0], BF16, kind="ExternalInput").ap()
    wqk_d = nc.dram_tensor("wqk_d", [128, CCH, 2 * CW], BF16, kind="ExternalInput").ap()
    wqk_c = nc.dram_tensor("wqk_c", [128, CCH, 2 * CW], BF16, kind="ExternalInput").ap()
    wv_d = nc.dram_tensor("wv_d", [128, CCH, CW], BF16, kind="ExternalInput").ap()
    wv_c = nc.dram_tensor("wv_c", [128, CCH, CW], BF16, kind="ExternalInput").ap()
    wp_d = nc.dram_tensor("wp_d", [128, CW // 128, C], BF16, kind="ExternalInput").ap()
    wp_c = nc.dram_tensor("wp_c", [128, CW // 128, C], BF16, kind="ExternalInput").ap()
    bias_d = nc.dram_tensor("bias_d", [1, CW], BF16, kind="ExternalInput").ap()
    bias_c = nc.dram_tensor("bias_c", [1, CW], BF16, kind="ExternalInput").ap()
    # o_d in natural [N, CW] layout (the transposed second attention
    # produces n-major chunks directly)
    o_d_nat = nc.dram_tensor("o_d_nat", [N, CW], BF16, kind="ExternalOutput").ap()
    o_c_full = nc.dram_tensor("o_c_cols", [N, CW], BF16, kind="ExternalOutput").ap()

    # ReduceScatter halves (bf16): input [2, NH, CW] slot-major; each pair
    # member receives its slot reduced: out [NH, CW].
    cc_in_d = [nc.dram_tensor(f"cc_in_d{k}", [2, NH, CW], BF16).ap() for k in range(2)]
    cc_out_d = [nc.dram_tensor(f"cc_out_d{k}", [NH, CW], BF16).ap() for k in range(2)]
    cc_in_c = [nc.dram_tensor(f"cc_in_c{k}", [2, NH, CW], BF16).ap() for k in range(2)]
    cc_out_c = [nc.dram_tensor(f"cc_out_c{k}", [NH, CW], BF16).ap() for k in range(2)]

    with tile.TileContext(nc) as tc:
        _body(nc, tc, xt, wqk_d, wqk_c, wv_d, wv_c, wp_d, wp_c,
              bias_d, bias_c, o_d_nat, o_c_full,
              cc_in_d, cc_out_d, cc_in_c, cc_out_c)
    nc.compile()
    return nc


def _body(nc, tc, xt, wqk_d, wqk_c, wv_d, wv_c, wp_d, wp_c,
          bias_d, bias_c, o_d_nat, o_c_full,
          cc_in_d, cc_out_d, cc_in_c, cc_out_c):
    from contextlib import ExitStack
    ctx = ExitStack()
    with ctx:
        ctx.enter_context(nc.allow_low_precision(reason="bf16 matmul inputs by design"))
        singles = ctx.enter_context(tc.tile_pool(name="singles", bufs=1))
        ident = singles.tile([128, 128], F32)
        make_identity(nc, ident[:])
        identb = singles.tile([128, 128], BF16)
        nc.vector.tensor_copy(identb[:], ident[:])

        big = ctx.enter_context(tc.tile_pool(name="big", bufs=1))
        qkT = {}    # branch -> [128, 6, N] bf16  rows: [q h0..h5 | k h0..h5]
        v_aug = {}  # branch -> [128, HPC, NCH, 65] bf16 (ones col -> row sums)
        u_t = {}    # branch -> [128, 3, N] bf16 (UNNORMALIZED (expS@v)^T)
        rT = {}     # recip row sums at partition 0, free-indexed by unit slot
        for br in ("d", "c"):
            v_aug[br] = big.tile([128, HPC, NCH, HD + 1], BF16, tag=f"v_{br}", name=f"v_{br}")
            nc.vector.memset(v_aug[br][:, :, :, HD:HD + 1], 1.0)
        qkT["c"] = big.tile([128, 2 * CW // 128, N], BF16, tag="qkT_c", name="qkT_c")
        u_t["c"] = big.tile([128, CW // 128, N], BF16, tag="u_c", name="u_c")
        u_t["d"] = big.tile([128, CW // 128, N], BF16, tag="u_d", name="u_d")
        # normalized AV output in n-major layout, staging for the transposes
        u_n = {}
        u_n["c"] = big.tile([128, NCH, HPC, HD], BF16, tag="un_c", name="un_c")
        u_n["d"] = big.tile([128, NCH, HPC, HD], BF16, tag="un_d", name="un_d")
        # per-partition reciprocal row sums [n-part, unit, chunk]; cond
        # persists into the o2 drain, diff is a small ring
        rT["c"] = big.tile([128, HPC * NB, 4], F32, tag="r_c", name="r_c")
        rT["d"] = big.tile([128, 4, 4], F32, tag="r_d", name="r_d")

        def rslot(br, h, nb):
            u = h * NB + nb
            return u % 4 if br == "d" else u

        wp_r = {}
        for br in ("d", "c"):
            wp_r[br] = big.tile([128, CW // 128, C], BF16, tag=f"wp_{br}", name=f"wp_{br}")
        bias_b = {}
        bias_b["d"] = big.tile([128, CW], BF16, tag="bias_bd", name="bias_bd")
        bias_b["c"] = big.tile([128, CW], BF16, tag="bias_bc", name="bias_bc")

        # ---------------- pools ----------------
        # qkT_d dies after the d-nb1 scores; the cond qkv load tiles die
        # after the mid-stream QKV_c; the cond eT ring reuses that region.
        dside = ctx.enter_context(tc.tile_pool(name="dside", bufs=1))
        qkT["d"] = dside.tile([128, 2 * CW // 128, N], BF16, tag="qkT_d", name="qkT_d")

        def eTd_pool_tile(tag, bufs, name):
            return dside.tile([128, NCH, 512], BF16, tag=tag, bufs=bufs, name=name)

        pj = ctx.enter_context(tc.tile_pool(name="pj", bufs=1))
        pdl = ctx.enter_context(tc.tile_pool(name="pdl", bufs=1))
        o2p = ctx.enter_context(tc.tile_pool(name="o2pool", bufs=2))
        ps_sc = ctx.enter_context(tc.tile_pool(name="ps_sc", bufs=2, space="PSUM"))
        ps_av = ctx.enter_context(tc.tile_pool(name="ps_av", bufs=2, space="PSUM"))
        ps_pj = ctx.enter_context(tc.tile_pool(name="ps_pj", bufs=2, space="PSUM"))

        # ---------------- emit helpers ----------------
        def load_qkv(br, wqk_r, wv_r, xT):
            wqk, wv = (wqk_d, wv_d) if br == "d" else (wqk_c, wv_c)
            half = 0 if br == "d" else N
            nc.sync.dma_start(out=wqk_r[:, :, 0:128], in_=wqk[:, :, 0:128])
            nc.sync.dma_start(out=xT[:, :, 0:512], in_=xt[:, :, half:half + 512])
            for f0 in range(128, 2 * CW, 320):
                f1 = min(f0 + 320, 2 * CW)
                nc.sync.dma_start(out=wqk_r[:, :, f0:f1], in_=wqk[:, :, f0:f1])
            nc.sync.dma_start(out=xT[:, :, 512:N], in_=xt[:, :, half + 512:half + N])
            nc.sync.dma_start(out=wv_r[:], in_=wv)

        def qkv_tasks(br, wqk_r, wv_r, xT):
            # QK: two fi-column groups per [128, 1024] scores-ring tile;
            # V: two m-chunks per tile.  Returns one closure per ring tile.
            def qk_pair(nb, fi):
                def emit():
                    ps = ps_sc.tile([128, 1024], F32, tag="sc_ps")
                    for k in range(2):
                        for ci in range(CCH):
                            nc.tensor.matmul(
                                ps[:, k * 512:(k + 1) * 512],
                                wqk_r[:, ci, (fi + k) * 128:(fi + k + 1) * 128],
                                xT[:, ci, nb * 512:(nb + 1) * 512],
                                start=(ci == 0), stop=(ci == CCH - 1))
                    nc.vector.tensor_copy(
                        qkT[br][:, fi:fi + 2, nb * 512:(nb + 1) * 512],
                        ps[:].rearrange("p (f q) -> p f q", f=2))
                return emit

            def v_pair(mch):
                def emit():
                    ps = ps_sc.tile([128, 1024], F32, tag="sc_ps")
                    for k in range(2):
                        for ci in range(CCH):
                            nc.tensor.matmul(
                                ps[:, k * 512:k * 512 + CW],
                                xT[:, ci, (mch + k) * 128:(mch + k + 1) * 128],
                                wv_r[:, ci, :],
                                start=(ci == 0), stop=(ci == CCH - 1))
                    for k in range(2):
                        nc.vector.tensor_copy(
                            v_aug[br][:, :, mch + k, 0:HD],
                            ps[:, k * 512:k * 512 + CW]
                            .rearrange("p (h d) -> p h d", h=HPC))
                return emit

            def qk_single(nb, fi):
                def emit():
                    ps = ps_sc.tile([128, 1024], F32, tag="sc_ps")
                    for ci in range(CCH):
                        nc.tensor.matmul(
                            ps[:, 0:512],
                            wqk_r[:, ci, fi * 128:(fi + 1) * 128],
                            xT[:, ci, nb * 512:(nb + 1) * 512],
                            start=(ci == 0), stop=(ci == CCH - 1))
                    nc.vector.tensor_copy(
                        qkT[br][:, fi, nb * 512:(nb + 1) * 512], ps[:, 0:512])
                return emit

            tasks = [qk_single(0, 0)]
            for nb in range(NB):
                start_fi = 1 if nb == 0 else 0
                for fi in range(start_fi, 2 * CW // 128 - 1, 2):
                    tasks.append(qk_pair(nb, fi))
                if (2 * CW // 128 - start_fi) % 2 == 1:
                    tasks.append(qk_single(nb, 2 * CW // 128 - 1))
            for mch in range(0, NCH, 2):
                tasks.append(v_pair(mch))
            return tasks

        def emit_scores(br, h, nb, eT_pool, tag, bufs):
            qc, qo = divmod(h * HD, 128)
            kc, ko = divmod(CW + h * HD, 128)
            eT = eT_pool.tile([128, NCH, 512], BF16, tag=tag, bufs=bufs,
                              name=f"eT_{br}_{h}_{nb}")
            for pr in range(NCH // 2):
                ps = ps_sc.tile([128, 1024], F32, tag="sc_ps")
                for k in range(2):
                    mch = pr * 2 + k
                    nc.tensor.matmul(
                        ps[:, k * 512:(k + 1) * 512],
                        qkT[br][ko:ko + HD, kc, mch * 128:(mch + 1) * 128],
                        qkT[br][qo:qo + HD, qc, nb * 512:(nb + 1) * 512],
                        start=True, stop=True)
                nc.scalar.activation(
                    eT[:, pr * 2:pr * 2 + 2, :].rearrange("p m q -> p (m q)"),
                    ps[:], Exp)
            return eT

        def emit_avmm(br, h, nb, eT):
            # TRANSPOSED AV: out[n, d] with eT as stationary, v as moving --
            # the moving width is only 65 (64 dims + ones column for the row
            # sums), so this runs at half the moving cost of the d-major
            # form.  Row sums land per-partition: normalization is a single
            # strided reciprocal + one broadcast multiply on DVE, no PE
            # expander needed.
            av = ps_av.tile([128, 4, HD + 1], F32, tag="av_ps")
            for j in range(4):
                for mch in range(NCH):
                    nc.tensor.matmul(
                        av[:, j, :],
                        eT[:, mch, j * 128:(j + 1) * 128],
                        v_aug[br][:, h, mch, :],
                        start=(mch == 0), stop=(mch == NCH - 1))
            slot = rslot(br, h, nb)
            nc.vector.reciprocal(rT[br][:, slot, :], av[:, :, HD])
            rb = rT[br][:, slot, :].rearrange("p (j o) -> p j o", o=1) \
                .broadcast_to([128, 4, HD])
            nc.vector.tensor_tensor(
                out=u_n[br][:, nb * 4:(nb + 1) * 4, h, :],
                in0=av[:, :, 0:HD], in1=rb, op=mybir.AluOpType.mult)
            return av

        def emit_fin(br, h, nb, av):
            # transpose the normalized n-major chunk into the d-major uT the
            # projection consumes (4 PE transposes + one DVE copy); the tp
            # tiles share the av psum ring (identical slot size)
            tp = ps_av.tile([HD, 4, 130], BF16, tag="av_ps")
            for j in range(4):
                nc.tensor.transpose(
                    tp[:, j, 0:128], u_n[br][:, nb * 4 + j, h, :], identb[:])
            uc, uo = divmod(h * HD, 128)
            nc.vector.tensor_copy(
                u_t[br][uo:uo + HD, uc, nb * 512:(nb + 1) * 512]
                .rearrange("p (j q) -> p j q", j=4),
                tp[:, :, 0:128])

        def proj_half(br, half):
            # bias/2 folded into each partial (the pair reduce completes it);
            # stores on DVE; one DMA per slot so the RS has few waits
            u = u_t[br]
            wp = wp_r[br]
            st = pj.tile([128, 2, 4, CW], BF16, tag="pj_st")
            cc = cc_in_d[half] if br == "d" else cc_in_c[half]
            # slot-major so slot 0's store DMA overlaps slot 1's matmuls:
            # the ReduceScatter emission chain shortens by one transfer
            for slot in range(2):
                for nch in range(half * 4, half * 4 + 4):
                    j = nch - half * 4
                    ps = ps_pj.tile([128, CW], F32, tag="pj_ps")
                    last = CW // 128 - 1
                    for ci in range(CW // 128):
                        nc.tensor.matmul(
                            ps[:],
                            u[:, ci, nch * 128:(nch + 1) * 128],
                            wp[:, ci, slot * CW:(slot + 1) * CW],
                            start=(ci == 0), stop=(ci == last))
                    nc.vector.tensor_add(st[:, slot, j, :], ps[:], bias_b[br][:])
                nc.sync.dma_start(
                    out=cc[slot].rearrange("(j p) f -> p j f", p=128),
                    in_=st[:, slot, :, :])
            nc.gpsimd.collective_compute(
                "ReduceScatter", AluOpType.add, replica_groups=GROUPS,
                ins=[cc], outs=[(cc_out_d if br == "d" else cc_out_c)[half]])

        pins = {}

        def pd_pin(k):
            # gpsimd SWDGE queue: this DMA waits on the diff ReduceScatter
            # and must not head-of-line block the SP store queue
            pin = pdl.tile([128, 4, CW], BF16, tag="p_in", bufs=2,
                           name=f"pin_{k}")
            nc.gpsimd.dma_start(
                out=pin[:],
                in_=cc_out_d[k].rearrange("(j p) f -> p j f", p=128))
            pins[k] = pin

        def emit_o2(h, nb, eT):
            # TRANSPOSED o2 = (attn_cond @ p_d) per head: eT stationary,
            # p_d columns moving (64 wide); normalized by the per-partition
            # r_c and stored straight into the natural o_d layout
            ps_o = ps_av.tile([128, 4, HD + 1], F32, tag="av_ps")
            for j in range(4):
                for mch in range(NCH):
                    pin = pins[0] if mch < 4 else pins[1]
                    nc.tensor.matmul(
                        ps_o[:, j, 0:HD],
                        eT[:, mch, j * 128:(j + 1) * 128],
                        pin[:, mch % 4, h * HD:(h + 1) * HD],
                        start=(mch == 0), stop=(mch == NCH - 1))
            rb = rT["c"][:, h * NB + nb, :].rearrange("p (j o) -> p j o", o=1) \
                .broadcast_to([128, 4, HD])
            o2T = o2p.tile([128, 4, HD], BF16, tag="o2T")
            nc.vector.tensor_tensor(out=o2T[:], in0=ps_o[:, :, 0:HD], in1=rb,
                                    op=mybir.AluOpType.mult)
            nc.sync.dma_start(
                out=o_d_nat[nb * 512:(nb + 1) * 512, h * HD:(h + 1) * HD]
                .rearrange("(j p) f -> p j f", p=128),
                in_=o2T[:])

        # ---------------- the stream ----------------
        # task stream with a trailing avmm (lag 1) / fin (lag 2) discipline;
        # fin(h5, nb) triggers that half's projection + ReduceScatter
        avq = []
        finq = []
        hooks = {}

        def advance(scored=None):
            if avq:
                br, h, nb, eT = avq.pop(0)
                finq.append((br, h, nb, emit_avmm(br, h, nb, eT)))
            if len(finq) > 1 or (scored is None and finq):
                br, h, nb, ps_u = finq.pop(0)
                emit_fin(br, h, nb, ps_u)
                if h == HPC - 1:
                    proj_half(br, nb)
                    hk = hooks.pop((br, nb), None)
                    if hk:
                        hk()
            if scored is not None:
                avq.append(scored)

        with tc.tile_pool(name="cond_ld", bufs=1) as cond_ld, \
             tc.tile_pool(name="dld", bufs=1) as dld:
            wqk_cr = cond_ld.tile([128, CCH, 2 * CW], BF16, tag="wqk_c", name="wqk_cr")
            wv_cr = cond_ld.tile([128, CCH, CW], BF16, tag="wv_c", name="wv_cr")
            xT_c = cond_ld.tile([128, CCH, N], BF16, tag="xT_c", name="xT_c")
            wqk_dr = dld.tile([128, CCH, 2 * CW], BF16, tag="wqk_d", name="wqk_dr")
            wv_dr = dld.tile([128, CCH, CW], BF16, tag="wv_d", name="wv_dr")
            xT_d = dld.tile([128, CCH, N], BF16, tag="xT_d", name="xT_d")

            # ---- loads: diff first (its QKV starts the kernel), cond +
            # proj weights + bias stream behind on the same queue ----
            load_qkv("d", wqk_dr, wv_dr, xT_d)
            load_qkv("c", wqk_cr, wv_cr, xT_c)
            for br in ("d", "c"):
                nc.sync.dma_start(out=wp_r[br][:], in_=wp_d if br == "d" else wp_c)
            nc.sync.dma_start(out=bias_b["d"][:], in_=bias_d.to_broadcast([128, CW]))
            nc.sync.dma_start(out=bias_b["c"][:], in_=bias_c.to_broadcast([128, CW]))

            # ---- QKV_d, then the diff attention with the cond QKV tasks
            # interleaved one-per-unit into the exp-bound units' PE slack ----
            for t in qkv_tasks("d", wqk_dr, wv_dr, xT_d):
                t()

            ctasks = qkv_tasks("c", wqk_cr, wv_cr, xT_c)
            ci = 0
            hooks[("d", 1)] = lambda: (pd_pin(0), pd_pin(1))
            for nb in range(NB):
                for h in range(HPC):
                    eT = emit_scores("d", h, nb, dside, "eT_d", 2)
                    advance(("d", h, nb, eT))
                    if nb + h > 0 and ci < len(ctasks):
                        ctasks[ci]()
                        ci += 1
            while ci < len(ctasks):
                ctasks[ci]()
                ci += 1
                advance()

        # load pools closed: the cond eT ring reuses their region.
        # group order c-nb0, d-nb1, c-nb1 packs the four ReduceScatters
        # back to back (d0 early, then c0/d1/c1 as their fins complete)
        with tc.tile_pool(name="eTc", bufs=1) as eTc:
            o2q = []
            for nb in range(NB):
                for h in range(HPC):
                    eT = emit_scores("c", h, nb, eTc, "eT_c", 2 * HPC)
                    o2q.append((h, nb, eT))
                    advance(("c", h, nb, eT))
            while avq or finq:
                advance()

            # ---- drain: all 12 second-attention units overlap the
            # last cond ReduceScatter ----
            nc.gpsimd.dma_start(out=o_c_full[0:NH, :], in_=cc_out_c[0])
            for h, nb, eT in o2q:
                emit_o2(h, nb, eT)
            nc.sync.dma_start(out=o_c_full[NH:NH + 256, :],
                              in_=cc_out_c[1][0:256, :])
            nc.gpsimd.dma_start(out=o_c_full[NH + 256:N, :],
                                in_=cc_out_c[1][256:NH, :])


def _prep_inputs(x, w_qkv_diff, w_qkv_cond, w_proj_diff, b_proj_diff,
                 w_proj_cond, b_proj_cond):
    import ml_dtypes
    bf = ml_dtypes.bfloat16
    in_maps = []
    wqk = {}
    wvv = {}
    for hg in range(2):
        s = slice(hg * CW, (hg + 1) * CW)
        sk = slice(C + hg * CW, C + (hg + 1) * CW)
        sv = slice(2 * C + hg * CW, 2 * C + (hg + 1) * CW)
        for br, w in (("d", w_qkv_diff), ("c", w_qkv_cond)):
            wqk[(br, hg)] = np.ascontiguousarray(
                np.concatenate([w[:, s] * SCALE, w[:, sk]], axis=1)
                .reshape(CCH, 128, 2 * CW).transpose(1, 0, 2).astype(bf))
            wvv[(br, hg)] = np.ascontiguousarray(
                w[:, sv].reshape(CCH, 128, CW).transpose(1, 0, 2).astype(bf))
    wp = {}
    for br, w in (("d", w_proj_diff), ("c", w_proj_cond)):
        for hg in range(2):
            s = slice(hg * CW, (hg + 1) * CW)
            wp[(br, hg)] = np.ascontiguousarray(
                w[s, :].reshape(CW // 128, 128, C).transpose(1, 0, 2).astype(bf))
    for c in range(N_CORES):
        b, hg = divmod(c, 2)
        s = slice(hg * CW, (hg + 1) * CW)
        m = {
            "xt": np.ascontiguousarray(
                x[b].T.reshape(CCH, 128, N0).transpose(1, 0, 2).astype(bf)),
            "wqk_d": wqk[("d", hg)],
            "wqk_c": wqk[("c", hg)],
            "wv_d": wvv[("d", hg)],
            "wv_c": wvv[("c", hg)],
            "wp_d": wp[("d", hg)],
            "wp_c": wp[("c", hg)],
            "bias_d": np.ascontiguousarray((0.5 * b_proj_diff[None, s]).astype(bf)),
            "bias_c": np.ascontiguousarray((0.5 * b_proj_cond[None, s]).astype(bf)),
        }
        in_maps.append(m)
    return in_maps


def kernel(x, w_qkv_diff, w_qkv_cond, w_proj_diff, b_proj_diff,
           w_proj_cond, b_proj_cond):
    x = np.asarray(x)
    w_qkv_diff = np.asarray(w_qkv_diff)
    w_qkv_cond = np.asarray(w_qkv_cond)
    w_proj_diff = np.asarray(w_proj_diff)
    b_proj_diff = np.asarray(b_proj_diff)
    w_proj_cond = np.asarray(w_proj_cond)
    b_proj_cond = np.asarray(b_proj_cond)

    if "nc" not in _CACHE:
        _CACHE["nc"] = _build()
    nc = _CACHE["nc"]
    in_maps = _prep_inputs(x, w_qkv_diff, w_qkv_cond, w_proj_diff,
                           b_proj_diff, w_proj_cond, b_proj_cond)
    res = run_bass_kernel_spmd(nc, in_maps, list(range(N_CORES))).results

    o_d = np.empty((B, N, C), np.float32)
    o_c = np.empty((B, N, C), np.float32)
    for c in range(N_CORES):
        b, hg = divmod(c, 2)
        o_d[b][:, hg * CW:(hg + 1) * CW] = np.asarray(res[c]["o_d_nat"]).astype(np.float32)
        o_c[b][:, hg * CW:(hg + 1) * CW] = np.asarray(res[c]["o_c_cols"]).astype(np.float32)
    return np.concatenate([o_d, o_c], axis=1)


# revision 65
# speedup vs baseline: 1.0043x; 1.0032x over previous
"""Trainium2 Bass kernel for nn_Attention_cross (dual-branch cross-reuse attention).

Reference computation (B=4, N0=2048, C=768, H=12, hd=64, N=1024):
  x_diff, x_cond = x[:, :N], x[:, N:]
  q,k,v per branch = x @ w_qkv (per-head), attn = softmax(q k^T / sqrt(hd))
  o_d = ((attn_diff @ v_d) @ w_proj_diff + b_d) reused per-head with attn_cond
  o_c = (attn_cond @ v_c) @ w_proj_cond + b_c
  out = concat([o_d, o_c], axis=1)

Sharding: 8 cores = 4 batches x 2 head-groups (6 heads each). The
head-mixing projections are row-sharded with bf16 pair ReduceScatters
(one per branch-half) whose serial 4x(15us+bytes) chain dominates the
schedule: the kernel is ordered [QKV_d, attn_d (with the cond QKV
tasks interleaved one-per-unit into the exp-bound units' PE slack),
attn_c nb0, attn_c nb1, o2-drain] so the first reduce is emitted as
early as possible and the four reduces run back to back behind
compute.  x arrives pre-transposed from the host so QKV starts
immediately with no on-chip transposes; QKV matmuls accumulate
straight into the scores PSUM ring.  Attention runs on exp(scores)
(softmax division deferred): the AV product is computed TRANSPOSED
(eT stationary, v moving with a ones column) so row sums land
per-partition and normalization is one strided reciprocal + one
stride-0-broadcast DVE multiply; PE transposes stage the result into
the d-major layout the projection consumes.  The second attention
(o2) is transposed the same way and written straight to the natural
o_d layout while the last ReduceScatter runs.  Projection partials
carry bias/2 each so the pair reduce completes the bias; p_d is
consumed directly from the reduce pins.  All matmul inputs bf16 (f32
PSUM accumulation); exp batched 2 PSUM banks per ACT instruction;
psum->SBUF copies and proj stores run on DVE keeping ACT pure-exp.
"""
import numpy as np

import concourse.bass as bass
import concourse.tile as tile
from concourse import bacc, mybir
from concourse.bass_utils import run_bass_kernel_spmd
from concourse.masks import make_identity
from concourse.alu_op_type import AluOpType

F32 = mybir.dt.float32
BF16 = mybir.dt.bfloat16
Exp = mybir.ActivationFunctionType.Exp
Copy = mybir.ActivationFunctionType.Copy

B, N0, C = 4, 2048, 768
H, HD = 12, 64
N = N0 // 2              # 1024 sequence per branch
HPC = H // 2             # 6 heads per core
CW = HPC * HD            # 384 own C-columns/rows
NCH = N // 128           # 8 chunks of 128 along n/m
CCH = C // 128           # 6 chunks of 128 along C
NB = N // 512            # 2 blocks of 512 along n
NH = N // 2              # 512 rows per RS chunk
SCALE = HD ** -0.5

N_CORES = 8
GROUPS = [[0, 1], [2, 3], [4, 5], [6, 7]]

_CACHE = {}


def _build():
    nc = bacc.Bacc("TRN2", target_bir_lowering=False, debug=False,
                   num_devices=N_CORES)

    # x arrives TRANSPOSED from host: [128, CCH, N0] (c-chunk-major cols)
    xt = nc.dram_tensor("xt", [128, CCH, N0], BF16, kind="ExternalInput").ap()
    wqk_d = nc.dram_tensor("wqk_d", [128, CCH, 2 * CW], BF16, kind="ExternalInput").ap()
    wqk_c = nc.dram_tensor("wqk_c", [128, CCH, 2 * CW], BF16, kind="ExternalInput").ap()
    wv_d = nc.dram_tensor("wv_d", [128, CCH, CW], BF16, kind="ExternalInput").ap()
    wv_c = nc.dram_tensor("wv_c", [128, CCH, CW], BF16, kind="ExternalInput").ap()
    wp_d = nc.dram_tensor("wp_d", [128, CW // 128, C], BF16, kind="ExternalInput").ap()
    wp_c = nc.dram_tensor("wp_c", [128, CW // 128, C], BF16, kind="ExternalInput").ap()
    bias_d = nc.dram_tensor("bias_d", [1, CW], BF16, kind="ExternalInput").ap()
    bias_c = nc.dram_tensor("bias_c", [1, CW], BF16, kind="ExternalInput").ap()
    # o_d in natural [N, CW] layout (the transposed second attention
    # produces n-major chunks directly)
    o_d_nat = nc.dram_tensor("o_d_nat", [N, CW], BF16, kind="ExternalOutput").ap()
    o_c_full = nc.dram_tensor("o_c_cols", [N, CW], BF16, kind="ExternalOutput").ap()

    # ReduceScatter halves (bf16): input [2, NH, CW] slot-major; each pair
    # member receives its slot reduced: out [NH, CW].
    cc_in_d = [nc.dram_tensor(f"cc_in_d{k}", [2, NH, CW], BF16).ap() for k in range(2)]
    cc_out_d = [nc.dram_tensor(f"cc_out_d{k}", [NH, CW], BF16).ap() for k in range(2)]
    cc_in_c = [nc.dram_tensor(f"cc_in_c{k}", [2, NH, CW], BF16).ap() for k in range(2)]
    cc_out_c = [nc.dram_tensor(f"cc_out_c{k}", [NH, CW], BF16).ap() for k in range(2)]

    with tile.TileContext(nc) as tc:
        _body(nc, tc, xt, wqk_d, wqk_c, wv_d, wv_c, wp_d, wp_c,
              bias_d, bias_c, o_d_nat, o_c_full,
              cc_in_d, cc_out_d, cc_in_c, cc_out_c)
    nc.compile()
    return nc


def _body(nc, tc, xt, wqk_d, wqk_c, wv_d, wv_c, wp_d, wp_c,
          bias_d, bias_c, o_d_nat, o_c_full,
          cc_in_d, cc_out_d, cc_in_c, cc_out_c):
    from contextlib import ExitStack
    ctx = ExitStack()
    with ctx:
        ctx.enter_context(nc.allow_low_precision(reason="bf16 matmul inputs by design"))
        singles = ctx.enter_context(tc.tile_pool(name="singles", bufs=1))
        ident = singles.tile([128, 128], F32)
        make_identity(nc, ident[:])
        identb = singles.tile([128, 128], BF16)
        nc.vector.tensor_copy(identb[:], ident[:])

        big = ctx.enter_context(tc.tile_pool(name="big", bufs=1))
        qkT = {}    # branch -> [128, 6, N] bf16  rows: [q h0..h5 | k h0..h5]
        v_aug = {}  # branch -> [128, HPC, NCH, 65] bf16 (ones col -> row sums)
        u_t = {}    # branch -> [128, 3, N] bf16 (UNNORMALIZED (expS@v)^T)
        rT = {}     # recip row sums at partition 0, free-indexed by unit slot
        for br in ("d", "c"):
            v_aug[br] = big.tile([128, HPC, NCH, HD + 1], BF16, tag=f"v_{br}", name=f"v_{br}")
            nc.vector.memset(v_aug[br][:, :, :, HD:HD + 1], 1.0)
        qkT["c"] = big.tile([128, 2 * CW // 128, N], BF16, tag="qkT_c", name="qkT_c")
        u_t["c"] = big.tile([128, CW // 128, N], BF16, tag="u_c", name="u_c")
        u_t["d"] = big.tile([128, CW // 128, N], BF16, tag="u_d", name="u_d")
        # normalized AV output in n-major layout, staging for the transposes
        u_n = {}
        u_n["c"] = big.tile([128, NCH, HPC, HD], BF16, tag="un_c", name="un_c")
        u_n["d"] = big.tile([128, NCH, HPC, HD], BF16, tag="un_d", name="un_d")
        # per-partition reciprocal row sums [n-part, unit, chunk]; cond
        # persists into the o2 drain, diff is a small ring
        rT["c"] = big.tile([128, HPC * NB, 4], F32, tag="r_c", name="r_c")
        rT["d"] = big.tile([128, 4, 4], F32, tag="r_d", name="r_d")

        def rslot(br, h, nb):
            u = h * NB + nb
            return u % 4 if br == "d" else u

        wp_r = {}
        for br in ("d", "c"):
            wp_r[br] = big.tile([128, CW // 128, C], BF16, tag=f"wp_{br}", name=f"wp_{br}")
        bias_b = {}
        bias_b["d"] = big.tile([128, CW], BF16, tag="bias_bd", name="bias_bd")
        bias_b["c"] = big.tile([128, CW], BF16, tag="bias_bc", name="bias_bc")

        # ---------------- pools ----------------
        # qkT_d dies after the d-nb1 scores; the cond qkv load tiles die
        # after the mid-stream QKV_c; the cond eT ring reuses that region.
        dside = ctx.enter_context(tc.tile_pool(name="dside", bufs=1))
        qkT["d"] = dside.tile([128, 2 * CW // 128, N], BF16, tag="qkT_d", name="qkT_d")

        def eTd_pool_tile(tag, bufs, name):
            return dside.tile([128, NCH, 512], BF16, tag=tag, bufs=bufs, name=name)

        pj = ctx.enter_context(tc.tile_pool(name="pj", bufs=1))
        pdl = ctx.enter_context(tc.tile_pool(name="pdl", bufs=1))
        o2p = ctx.enter_context(tc.tile_pool(name="o2pool", bufs=2))
        ps_sc = ctx.enter_context(tc.tile_pool(name="ps_sc", bufs=2, space="PSUM"))
        ps_av = ctx.enter_context(tc.tile_pool(name="ps_av", bufs=2, space="PSUM"))
        ps_pj = ctx.enter_context(tc.tile_pool(name="ps_pj", bufs=2, space="PSUM"))

        # ---------------- emit helpers ----------------
        def load_qkv(br, wqk_r, wv_r, xT):
            wqk, wv = (wqk_d, wv_d) if br == "d" else (wqk_c, wv_c)
            half = 0 if br == "d" else N
            nc.sync.dma_start(out=wqk_r[:, :, 0:128], in_=wqk[:, :, 0:128])
            nc.sync.dma_start(out=xT[:, :, 0:512], in_=xt[:, :, half:half + 512])
            for f0 in range(128, 2 * CW, 320):
                f1 = min(f0 + 320, 2 * CW)
                nc.sync.dma_start(out=wqk_r[:, :, f0:f1], in_=wqk[:, :, f0:f1])
            nc.sync.dma_start(out=xT[:, :, 512:N], in_=xt[:, :, half + 512:half + N])
            nc.sync.dma_start(out=wv_r[:], in_=wv)

        def qkv_tasks(br, wqk_r, wv_r, xT):
            # QK: two fi-column groups per [128, 1024] scores-ring tile;
            # V: two m-chunks per tile.  Returns one closure per ring tile.
            def qk_pair(nb, fi):
                def emit():
                    ps = ps_sc.tile([128, 1024], F32, tag="sc_ps")
                    for k in range(2):
                        for ci in range(CCH):
                            nc.tensor.matmul(
                                ps[:, k * 512:(k + 1) * 512],
                                wqk_r[:, ci, (fi + k) * 128:(fi + k + 1) * 128],
                                xT[:, ci, nb * 512:(nb + 1) * 512],
                                start=(ci == 0), stop=(ci == CCH - 1))
                    nc.vector.tensor_copy(
                        qkT[br][:, fi:fi + 2, nb * 512:(nb + 1) * 512],
                        ps[:].rearrange("p (f q) -> p f q", f=2))
                return emit

            def v_pair(mch):
                def emit():
                    ps = ps_sc.tile([128, 1024], F32, tag="sc_ps")
                    for k in range(2):
                        for ci in range(CCH):
                            nc.tensor.matmul(
                                ps[:, k * 512:k * 512 + CW],
                                xT[:, ci, (mch + k) * 128:(mch + k + 1) * 128],
                                wv_r[:, ci, :],
                                start=(ci == 0), stop=(ci == CCH - 1))
                    for k in range(2):
                        nc.vector.tensor_copy(
                            v_aug[br][:, :, mch + k, 0:HD],
                            ps[:, k * 512:k * 512 + CW]
                            .rearrange("p (h d) -> p h d", h=HPC))
                return emit

            def qk_single(nb, fi):
                def emit():
                    ps = ps_sc.tile([128, 1024], F32, tag="sc_ps")
                    for ci in range(CCH):
                        nc.tensor.matmul(
                            ps[:, 0:512],
                            wqk_r[:, ci, fi * 128:(fi + 1) * 128],
                            xT[:, ci, nb * 512:(nb + 1) * 512],
                            start=(ci == 0), stop=(ci == CCH - 1))
                    nc.vector.tensor_copy(
                        qkT[br][:, fi, nb * 512:(nb + 1) * 512], ps[:, 0:512])
                return emit

            tasks = [qk_single(0, 0)]
            for nb in range(NB):
                start_fi = 1 if nb == 0 else 0
                for fi in range(start_fi, 2 * CW // 128 - 1, 2):
                    tasks.append(qk_pair(nb, fi))
                if (2 * CW // 128 - start_fi) % 2 == 1:
                    tasks.append(qk_single(nb, 2 * CW // 128 - 1))
            for mch in range(0, NCH, 2):
                tasks.append(v_pair(mch))
            return tasks

        def emit_scores(br, h, nb, eT_pool, tag, bufs):
            qc, qo = divmod(h * HD, 128)
            kc, ko = divmod(CW + h * HD, 128)
            eT = eT_pool.tile([128, NCH, 512], BF16, tag=tag, bufs=bufs,
                              name=f"eT_{br}_{h}_{nb}")
            for pr in range(NCH // 2):
                ps = ps_sc.tile([128, 1024], F32, tag="sc_ps")
                for k in range(2):
                    mch = pr * 2 + k
                    nc.tensor.matmul(
                        ps[:, k * 512:(k + 1) * 512],
                        qkT[br][ko:ko + HD, kc, mch * 128:(mch + 1) * 128],
                        qkT[br][qo:qo + HD, qc, nb * 512:(nb + 1) * 512],
                        start=True, stop=True)
                nc.scalar.activation(
                    eT[:, pr * 2:pr * 2 + 2, :].rearrange("p m q -> p (m q)"),
                    ps[:], Exp)
            return eT

        def emit_avmm(br, h, nb, eT):
            # TRANSPOSED AV: out[n, d] with eT as stationary, v as moving --
            # the moving width is only 65 (64 dims + ones column for the row
            # sums), so this runs at half the moving cost of the d-major
            # form.  Row sums land per-partition: normalization is a single
            # strided reciprocal + one broadcast multiply on DVE, no PE
            # expander needed.
            av = ps_av.tile([128, 4, HD + 1], F32, tag="av_ps")
            for j in range(4):
                for mch in range(NCH):
                    nc.tensor.matmul(
                        av[:, j, :],
                        eT[:, mch, j * 128:(j + 1) * 128],
                        v_aug[br][:, h, mch, :],
                        start=(mch == 0), stop=(mch == NCH - 1))
            slot = rslot(br, h, nb)
            nc.vector.reciprocal(rT[br][:, slot, :], av[:, :, HD])
            rb = rT[br][:, slot, :].rearrange("p (j o) -> p j o", o=1) \
                .broadcast_to([128, 4, HD])
            nc.vector.tensor_tensor(
                out=u_n[br][:, nb * 4:(nb + 1) * 4, h, :],
                in0=av[:, :, 0:HD], in1=rb, op=mybir.AluOpType.mult)
            return av

        def emit_fin(br, h, nb, av):
            # transpose the normalized n-major chunk into the d-major uT the
            # projection consumes (4 PE transposes + one DVE copy); the tp
            # tiles share the av psum ring (identical slot size)
            tp = ps_av.tile([HD, 4, 130], BF16, tag="av_ps")
            for j in range(4):
                nc.tensor.transpose(
                    tp[:, j, 0:128], u_n[br][:, nb * 4 + j, h, :], identb[:])
            uc, uo = divmod(h * HD, 128)
            nc.vector.tensor_copy(
                u_t[br][uo:uo + HD, uc, nb * 512:(nb + 1) * 512]
                .rearrange("p (j q) -> p j q", j=4),
                tp[:, :, 0:128])

        def proj_half(br, half):
            # bias/2 folded into each partial (the pair reduce completes it);
            # stores on DVE; one DMA per slot so the RS has few waits
            u = u_t[br]
            wp = wp_r[br]
            st = pj.tile([128, 2, 4, CW], BF16, tag="pj_st")
            cc = cc_in_d[half] if br == "d" else cc_in_c[half]
            # slot-major so slot 0's store DMA overlaps slot 1's matmuls:
            # the ReduceScatter emission chain shortens by one transfer
            for slot in range(2):
                for nch in range(half * 4, half * 4 + 4):
                    j = nch - half * 4
                    ps = ps_pj.tile([128, CW], F32, tag="pj_ps")
                    last = CW // 128 - 1
                    for ci in range(CW // 128):
                        nc.tensor.matmul(
                            ps[:],
                            u[:, ci, nch * 128:(nch + 1) * 128],
                            wp[:, ci, slot * CW:(slot + 1) * CW],
                            start=(ci == 0), stop=(ci == last))
                    nc.vector.tensor_add(st[:, slot, j, :], ps[:], bias_b[br][:])
                    if j == 1:
                        nc.sync.dma_start(
                            out=cc[slot][0:256].rearrange("(j p) f -> p j f", p=128),
                            in_=st[:, slot, 0:2, :])
                nc.sync.dma_start(
                    out=cc[slot][256:NH].rearrange("(j p) f -> p j f", p=128),
                    in_=st[:, slot, 2:4, :])
            nc.gpsimd.collective_compute(
                "ReduceScatter", AluOpType.add, replica_groups=GROUPS,
                ins=[cc], outs=[(cc_out_d if br == "d" else cc_out_c)[half]])

        pins = {}

        def pd_pin(k):
            # gpsimd SWDGE queue: this DMA waits on the diff ReduceScatter
            # and must not head-of-line block the SP store queue
            pin = pdl.tile([128, 4, CW], BF16, tag="p_in", bufs=2,
                           name=f"pin_{k}")
            nc.gpsimd.dma_start(
                out=pin[:],
                in_=cc_out_d[k].rearrange("(j p) f -> p j f", p=128))
            pins[k] = pin

        def emit_o2(h, nb, eT):
            # TRANSPOSED o2 = (attn_cond @ p_d) per head: eT stationary,
            # p_d columns moving (64 wide); normalized by the per-partition
            # r_c and stored straight into the natural o_d layout
            ps_o = ps_av.tile([128, 4, HD + 1], F32, tag="av_ps")
            for j in range(4):
                for mch in range(NCH):
                    pin = pins[0] if mch < 4 else pins[1]
                    nc.tensor.matmul(
                        ps_o[:, j, 0:HD],
                        eT[:, mch, j * 128:(j + 1) * 128],
                        pin[:, mch % 4, h * HD:(h + 1) * HD],
                        start=(mch == 0), stop=(mch == NCH - 1))
            rb = rT["c"][:, h * NB + nb, :].rearrange("p (j o) -> p j o", o=1) \
                .broadcast_to([128, 4, HD])
            o2T = o2p.tile([128, 4, HD], BF16, tag="o2T")
            nc.vector.tensor_tensor(out=o2T[:], in0=ps_o[:, :, 0:HD], in1=rb,
                                    op=mybir.AluOpType.mult)
            nc.sync.dma_start(
                out=o_d_nat[nb * 512:(nb + 1) * 512, h * HD:(h + 1) * HD]
                .rearrange("(j p) f -> p j f", p=128),
                in_=o2T[:])

        # ---------------- the stream ----------------
        # task stream with a trailing avmm (lag 1) / fin (lag 2) discipline;
        # fin(h5, nb) triggers that half's projection + ReduceScatter
        avq = []
        finq = []
        hooks = {}

        def advance(scored=None):
            if avq:
                br, h, nb, eT = avq.pop(0)
                finq.append((br, h, nb, emit_avmm(br, h, nb, eT)))
            if len(finq) > 1 or (scored is None and finq):
                br, h, nb, ps_u = finq.pop(0)
                emit_fin(br, h, nb, ps_u)
                if h == HPC - 1:
                    proj_half(br, nb)
                    hk = hooks.pop((br, nb), None)
                    if hk:
                        hk()
            if scored is not None:
                avq.append(scored)

        with tc.tile_pool(name="cond_ld", bufs=1) as cond_ld, \
             tc.tile_pool(name="dld", bufs=1) as dld:
            wqk_cr = cond_ld.tile([128, CCH, 2 * CW], BF16, tag="wqk_c", name="wqk_cr")
            wv_cr = cond_ld.tile([128, CCH, CW], BF16, tag="wv_c", name="wv_cr")
            xT_c = cond_ld.tile([128, CCH, N], BF16, tag="xT_c", name="xT_c")
            wqk_dr = dld.tile([128, CCH, 2 * CW], BF16, tag="wqk_d", name="wqk_dr")
            wv_dr = dld.tile([128, CCH, CW], BF16, tag="wv_d", name="wv_dr")
            xT_d = dld.tile([128, CCH, N], BF16, tag="xT_d", name="xT_d")

            # ---- loads: diff first (its QKV starts the kernel), cond +
            # proj weights + bias stream behind on the same queue ----
            load_qkv("d", wqk_dr, wv_dr, xT_d)
            load_qkv("c", wqk_cr, wv_cr, xT_c)
            for br in ("d", "c"):
                nc.sync.dma_start(out=wp_r[br][:], in_=wp_d if br == "d" else wp_c)
            nc.sync.dma_start(out=bias_b["d"][:], in_=bias_d.to_broadcast([128, CW]))
            nc.sync.dma_start(out=bias_b["c"][:], in_=bias_c.to_broadcast([128, CW]))

            # ---- QKV_d, then the diff attention with the cond QKV tasks
            # interleaved one-per-unit into the exp-bound units' PE slack ----
            for t in qkv_tasks("d", wqk_dr, wv_dr, xT_d):
                t()

            ctasks = qkv_tasks("c", wqk_cr, wv_cr, xT_c)
            ci = 0
            hooks[("d", 1)] = lambda: (pd_pin(0), pd_pin(1))
            for nb in range(NB):
                for h in range(HPC):
                    eT = emit_scores("d", h, nb, dside, "eT_d", 2)
                    advance(("d", h, nb, eT))
                    if nb + h > 0 and ci < len(ctasks):
                        ctasks[ci]()
                        ci += 1
            while ci < len(ctasks):
                ctasks[ci]()
                ci += 1
                advance()

        # load pools closed: the cond eT ring reuses their region.
        # group order c-nb0, d-nb1, c-nb1 packs the four ReduceScatters
        # back to back (d0 early, then c0/d1/c1 as their fins complete)
        with tc.tile_pool(name="eTc", bufs=1) as eTc:
            o2q = []
            for nb in range(NB):
                for h in range(HPC):
                    eT = emit_scores("c", h, nb, eTc, "eT_c", 2 * HPC)
                    o2q.append((h, nb, eT))
                    advance(("c", h, nb, eT))
            while avq or finq:
                advance()

            # ---- drain: all 12 second-attention units overlap the
            # last cond ReduceScatter ----
            nc.gpsimd.dma_start(out=o_c_full[0:NH, :], in_=cc_out_c[0])
            for h, nb, eT in o2q:
                emit_o2(h, nb, eT)
            nc.sync.dma_start(out=o_c_full[NH:NH + 176, :],
                              in_=cc_out_c[1][0:176, :])
            nc.scalar.dma_start(out=o_c_full[NH + 176:NH + 352, :],
                                in_=cc_out_c[1][176:352, :])
            nc.gpsimd.dma_start(out=o_c_full[NH + 352:N, :],
                                in_=cc_out_c[1][352:NH, :])


def _prep_inputs(x, w_qkv_diff, w_qkv_cond, w_proj_diff, b_proj_diff,
                 w_proj_cond, b_proj_cond):
    import ml_dtypes
    bf = ml_dtypes.bfloat16
    in_maps = []
    wqk = {}
    wvv = {}
    for hg in range(2):
        s = slice(hg * CW, (hg + 1) * CW)
        sk = slice(C + hg * CW, C + (hg + 1) * CW)
        sv = slice(2 * C + hg * CW, 2 * C + (hg + 1) * CW)
        for br, w in (("d", w_qkv_diff), ("c", w_qkv_cond)):
            wqk[(br, hg)] = np.ascontiguousarray(
                np.concatenate([w[:, s] * SCALE, w[:, sk]], axis=1)
                .reshape(CCH, 128, 2 * CW).transpose(1, 0, 2).astype(bf))
            wvv[(br, hg)] = np.ascontiguousarray(
                w[:, sv].reshape(CCH, 128, CW).transpose(1, 0, 2).astype(bf))
    wp = {}
    for br, w in (("d", w_proj_diff), ("c", w_proj_cond)):
        for hg in range(2):
            s = slice(hg * CW, (hg + 1) * CW)
            wp[(br, hg)] = np.ascontiguousarray(
                w[s, :].reshape(CW // 128, 128, C).transpose(1, 0, 2).astype(bf))
    for c in range(N_CORES):
        b, hg = divmod(c, 2)
        s = slice(hg * CW, (hg + 1) * CW)
        m = {
            "xt": np.ascontiguousarray(
                x[b].T.reshape(CCH, 128, N0).transpose(1, 0, 2).astype(bf)),
            "wqk_d": wqk[("d", hg)],
            "wqk_c": wqk[("c", hg)],
            "wv_d": wvv[("d", hg)],
            "wv_c": wvv[("c", hg)],
            "wp_d": wp[("d", hg)],
            "wp_c": wp[("c", hg)],
            "bias_d": np.ascontiguousarray((0.5 * b_proj_diff[None, s]).astype(bf)),
            "bias_c": np.ascontiguousarray((0.5 * b_proj_cond[None, s]).astype(bf)),
        }
        in_maps.append(m)
    return in_maps


def kernel(x, w_qkv_diff, w_qkv_cond, w_proj_diff, b_proj_diff,
           w_proj_cond, b_proj_cond):
    x = np.asarray(x)
    w_qkv_diff = np.asarray(w_qkv_diff)
    w_qkv_cond = np.asarray(w_qkv_cond)
    w_proj_diff = np.asarray(w_proj_diff)
    b_proj_diff = np.asarray(b_proj_diff)
    w_proj_cond = np.asarray(w_proj_cond)
    b_proj_cond = np.asarray(b_proj_cond)

    if "nc" not in _CACHE:
        _CACHE["nc"] = _build()
    nc = _CACHE["nc"]
    in_maps = _prep_inputs(x, w_qkv_diff, w_qkv_cond, w_proj_diff,
                           b_proj_diff, w_proj_cond, b_proj_cond)
    res = run_bass_kernel_spmd(nc, in_maps, list(range(N_CORES))).results

    o_d = np.empty((B, N, C), np.float32)
    o_c = np.empty((B, N, C), np.float32)
    for c in range(N_CORES):
        b, hg = divmod(c, 2)
        o_d[b][:, hg * CW:(hg + 1) * CW] = np.asarray(res[c]["o_d_nat"]).astype(np.float32)
        o_c[b][:, hg * CW:(hg + 1) * CW] = np.asarray(res[c]["o_c_cols"]).astype(np.float32)
    return np.concatenate([o_d, o_c], axis=1)


# revision 66
# speedup vs baseline: 1.0053x; 1.0010x over previous
"""Trainium2 Bass kernel for nn_Attention_cross (dual-branch cross-reuse attention).

Reference computation (B=4, N0=2048, C=768, H=12, hd=64, N=1024):
  x_diff, x_cond = x[:, :N], x[:, N:]
  q,k,v per branch = x @ w_qkv (per-head), attn = softmax(q k^T / sqrt(hd))
  o_d = ((attn_diff @ v_d) @ w_proj_diff + b_d) reused per-head with attn_cond
  o_c = (attn_cond @ v_c) @ w_proj_cond + b_c
  out = concat([o_d, o_c], axis=1)

Sharding: 8 cores = 4 batches x 2 head-groups (6 heads each). The
head-mixing projections are row-sharded with bf16 pair ReduceScatters
(one per branch-half) whose serial 4x(15us+bytes) chain dominates the
schedule: the kernel is ordered [QKV_d, attn_d (with the cond QKV
tasks interleaved one-per-unit into the exp-bound units' PE slack),
attn_c nb0, attn_c nb1, o2-drain] so the first reduce is emitted as
early as possible and the four reduces run back to back behind
compute.  x arrives pre-transposed from the host so QKV starts
immediately with no on-chip transposes; QKV matmuls accumulate
straight into the scores PSUM ring.  Attention runs on exp(scores)
(softmax division deferred): the AV product is computed TRANSPOSED
(eT stationary, v moving with a ones column) so row sums land
per-partition and normalization is one strided reciprocal + one
stride-0-broadcast DVE multiply; PE transposes stage the result into
the d-major layout the projection consumes.  The second attention
(o2) is transposed the same way and written straight to the natural
o_d layout while the last ReduceScatter runs.  Projection partials
carry bias/2 each so the pair reduce completes the bias; p_d is
consumed directly from the reduce pins.  All matmul inputs bf16 (f32
PSUM accumulation); exp batched 2 PSUM banks per ACT instruction;
psum->SBUF copies and proj stores run on DVE keeping ACT pure-exp.
"""
import numpy as np

import concourse.bass as bass
import concourse.tile as tile
from concourse import bacc, mybir
from concourse.bass_utils import run_bass_kernel_spmd
from concourse.masks import make_identity
from concourse.alu_op_type import AluOpType

F32 = mybir.dt.float32
BF16 = mybir.dt.bfloat16
Exp = mybir.ActivationFunctionType.Exp
Copy = mybir.ActivationFunctionType.Copy

B, N0, C = 4, 2048, 768
H, HD = 12, 64
N = N0 // 2              # 1024 sequence per branch
HPC = H // 2             # 6 heads per core
CW = HPC * HD            # 384 own C-columns/rows
NCH = N // 128           # 8 chunks of 128 along n/m
CCH = C // 128           # 6 chunks of 128 along C
NB = N // 512            # 2 blocks of 512 along n
NH = N // 2              # 512 rows per RS chunk
SCALE = HD ** -0.5

N_CORES = 8
GROUPS = [[0, 1], [2, 3], [4, 5], [6, 7]]

_CACHE = {}


def _build():
    nc = bacc.Bacc("TRN2", target_bir_lowering=False, debug=False,
                   num_devices=N_CORES)

    # x arrives TRANSPOSED from host: [128, CCH, N0] (c-chunk-major cols)
    xt = nc.dram_tensor("xt", [128, CCH, N0], BF16, kind="ExternalInput").ap()
    wqk_d = nc.dram_tensor("wqk_d", [128, CCH, 2 * CW], BF16, kind="ExternalInput").ap()
    wqk_c = nc.dram_tensor("wqk_c", [128, CCH, 2 * CW], BF16, kind="ExternalInput").ap()
    wv_d = nc.dram_tensor("wv_d", [128, CCH, CW], BF16, kind="ExternalInput").ap()
    wv_c = nc.dram_tensor("wv_c", [128, CCH, CW], BF16, kind="ExternalInput").ap()
    wp_d = nc.dram_tensor("wp_d", [128, CW // 128, C], BF16, kind="ExternalInput").ap()
    wp_c = nc.dram_tensor("wp_c", [128, CW // 128, C], BF16, kind="ExternalInput").ap()
    bias_d = nc.dram_tensor("bias_d", [1, CW], BF16, kind="ExternalInput").ap()
    bias_c = nc.dram_tensor("bias_c", [1, CW], BF16, kind="ExternalInput").ap()
    # o_d in natural [N, CW] layout (the transposed second attention
    # produces n-major chunks directly)
    o_d_nat = nc.dram_tensor("o_d_nat", [N, CW], BF16, kind="ExternalOutput").ap()
    o_c_full = nc.dram_tensor("o_c_cols", [N, CW], BF16, kind="ExternalOutput").ap()

    # ReduceScatter halves (bf16): input [2, NH, CW] slot-major; each pair
    # member receives its slot reduced: out [NH, CW].
    cc_in_d = [nc.dram_tensor(f"cc_in_d{k}", [2, NH, CW], BF16).ap() for k in range(2)]
    cc_out_d = [nc.dram_tensor(f"cc_out_d{k}", [NH, CW], BF16).ap() for k in range(2)]
    cc_in_c = [nc.dram_tensor(f"cc_in_c{k}", [2, NH, CW], BF16).ap() for k in range(2)]
    cc_out_c = [nc.dram_tensor(f"cc_out_c{k}", [NH, CW], BF16).ap() for k in range(2)]

    with tile.TileContext(nc) as tc:
        _body(nc, tc, xt, wqk_d, wqk_c, wv_d, wv_c, wp_d, wp_c,
              bias_d, bias_c, o_d_nat, o_c_full,
              cc_in_d, cc_out_d, cc_in_c, cc_out_c)
    nc.compile()
    return nc


def _body(nc, tc, xt, wqk_d, wqk_c, wv_d, wv_c, wp_d, wp_c,
          bias_d, bias_c, o_d_nat, o_c_full,
          cc_in_d, cc_out_d, cc_in_c, cc_out_c):
    from contextlib import ExitStack
    ctx = ExitStack()
    with ctx:
        ctx.enter_context(nc.allow_low_precision(reason="bf16 matmul inputs by design"))
        singles = ctx.enter_context(tc.tile_pool(name="singles", bufs=1))
        ident = singles.tile([128, 128], F32)
        make_identity(nc, ident[:])
        identb = singles.tile([128, 128], BF16)
        nc.vector.tensor_copy(identb[:], ident[:])

        big = ctx.enter_context(tc.tile_pool(name="big", bufs=1))
        qkT = {}    # branch -> [128, 6, N] bf16  rows: [q h0..h5 | k h0..h5]
        v_aug = {}  # branch -> [128, HPC, NCH, 65] bf16 (ones col -> row sums)
        u_t = {}    # branch -> [128, 3, N] bf16 (UNNORMALIZED (expS@v)^T)
        rT = {}     # recip row sums at partition 0, free-indexed by unit slot
        for br in ("d", "c"):
            v_aug[br] = big.tile([128, HPC, NCH, HD + 1], BF16, tag=f"v_{br}", name=f"v_{br}")
            nc.vector.memset(v_aug[br][:, :, :, HD:HD + 1], 1.0)
        qkT["c"] = big.tile([128, 2 * CW // 128, N], BF16, tag="qkT_c", name="qkT_c")
        u_t["c"] = big.tile([128, CW // 128, N], BF16, tag="u_c", name="u_c")
        u_t["d"] = big.tile([128, CW // 128, N], BF16, tag="u_d", name="u_d")
        # normalized AV output in n-major layout, staging for the transposes
        u_n = {}
        u_n["c"] = big.tile([128, NCH, HPC, HD], BF16, tag="un_c", name="un_c")
        u_n["d"] = big.tile([128, NCH, HPC, HD], BF16, tag="un_d", name="un_d")
        # per-partition reciprocal row sums [n-part, unit, chunk]; cond
        # persists into the o2 drain, diff is a small ring
        rT["c"] = big.tile([128, HPC * NB, 4], F32, tag="r_c", name="r_c")
        rT["d"] = big.tile([128, 4, 4], F32, tag="r_d", name="r_d")

        def rslot(br, h, nb):
            u = h * NB + nb
            return u % 4 if br == "d" else u

        wp_r = {}
        for br in ("d", "c"):
            wp_r[br] = big.tile([128, CW // 128, C], BF16, tag=f"wp_{br}", name=f"wp_{br}")
        bias_b = {}
        bias_b["d"] = big.tile([128, CW], BF16, tag="bias_bd", name="bias_bd")
        bias_b["c"] = big.tile([128, CW], BF16, tag="bias_bc", name="bias_bc")

        # ---------------- pools ----------------
        # qkT_d dies after the d-nb1 scores; the cond qkv load tiles die
        # after the mid-stream QKV_c; the cond eT ring reuses that region.
        dside = ctx.enter_context(tc.tile_pool(name="dside", bufs=1))
        qkT["d"] = dside.tile([128, 2 * CW // 128, N], BF16, tag="qkT_d", name="qkT_d")

        def eTd_pool_tile(tag, bufs, name):
            return dside.tile([128, NCH, 512], BF16, tag=tag, bufs=bufs, name=name)

        pj = ctx.enter_context(tc.tile_pool(name="pj", bufs=1))
        pdl = ctx.enter_context(tc.tile_pool(name="pdl", bufs=1))
        o2p = ctx.enter_context(tc.tile_pool(name="o2pool", bufs=2))
        ps_sc = ctx.enter_context(tc.tile_pool(name="ps_sc", bufs=2, space="PSUM"))
        ps_av = ctx.enter_context(tc.tile_pool(name="ps_av", bufs=2, space="PSUM"))
        ps_pj = ctx.enter_context(tc.tile_pool(name="ps_pj", bufs=2, space="PSUM"))

        # ---------------- emit helpers ----------------
        def load_qkv(br, wqk_r, wv_r, xT):
            wqk, wv = (wqk_d, wv_d) if br == "d" else (wqk_c, wv_c)
            half = 0 if br == "d" else N
            nc.sync.dma_start(out=wqk_r[:, :, 0:128], in_=wqk[:, :, 0:128])
            nc.sync.dma_start(out=xT[:, :, 0:512], in_=xt[:, :, half:half + 512])
            for f0 in range(128, 2 * CW, 320):
                f1 = min(f0 + 320, 2 * CW)
                nc.sync.dma_start(out=wqk_r[:, :, f0:f1], in_=wqk[:, :, f0:f1])
            nc.sync.dma_start(out=xT[:, :, 512:N], in_=xt[:, :, half + 512:half + N])
            nc.sync.dma_start(out=wv_r[:], in_=wv)

        def qkv_tasks(br, wqk_r, wv_r, xT):
            # QK: two fi-column groups per [128, 1024] scores-ring tile;
            # V: two m-chunks per tile.  Returns one closure per ring tile.
            def qk_pair(nb, fi):
                def emit():
                    ps = ps_sc.tile([128, 1024], F32, tag="sc_ps")
                    for k in range(2):
                        for ci in range(CCH):
                            nc.tensor.matmul(
                                ps[:, k * 512:(k + 1) * 512],
                                wqk_r[:, ci, (fi + k) * 128:(fi + k + 1) * 128],
                                xT[:, ci, nb * 512:(nb + 1) * 512],
                                start=(ci == 0), stop=(ci == CCH - 1))
                    nc.vector.tensor_copy(
                        qkT[br][:, fi:fi + 2, nb * 512:(nb + 1) * 512],
                        ps[:].rearrange("p (f q) -> p f q", f=2))
                return emit

            def v_pair(mch):
                def emit():
                    ps = ps_sc.tile([128, 1024], F32, tag="sc_ps")
                    for k in range(2):
                        for ci in range(CCH):
                            nc.tensor.matmul(
                                ps[:, k * 512:k * 512 + CW],
                                xT[:, ci, (mch + k) * 128:(mch + k + 1) * 128],
                                wv_r[:, ci, :],
                                start=(ci == 0), stop=(ci == CCH - 1))
                    for k in range(2):
                        nc.vector.tensor_copy(
                            v_aug[br][:, :, mch + k, 0:HD],
                            ps[:, k * 512:k * 512 + CW]
                            .rearrange("p (h d) -> p h d", h=HPC))
                return emit

            def qk_single(nb, fi):
                def emit():
                    ps = ps_sc.tile([128, 1024], F32, tag="sc_ps")
                    for ci in range(CCH):
                        nc.tensor.matmul(
                            ps[:, 0:512],
                            wqk_r[:, ci, fi * 128:(fi + 1) * 128],
                            xT[:, ci, nb * 512:(nb + 1) * 512],
                            start=(ci == 0), stop=(ci == CCH - 1))
                    nc.vector.tensor_copy(
                        qkT[br][:, fi, nb * 512:(nb + 1) * 512], ps[:, 0:512])
                return emit

            tasks = [qk_single(0, 0)]
            for nb in range(NB):
                start_fi = 1 if nb == 0 else 0
                for fi in range(start_fi, 2 * CW // 128 - 1, 2):
                    tasks.append(qk_pair(nb, fi))
                if (2 * CW // 128 - start_fi) % 2 == 1:
                    tasks.append(qk_single(nb, 2 * CW // 128 - 1))
            for mch in range(0, NCH, 2):
                tasks.append(v_pair(mch))
            return tasks

        def emit_scores(br, h, nb, eT_pool, tag, bufs):
            qc, qo = divmod(h * HD, 128)
            kc, ko = divmod(CW + h * HD, 128)
            eT = eT_pool.tile([128, NCH, 512], BF16, tag=tag, bufs=bufs,
                              name=f"eT_{br}_{h}_{nb}")
            for pr in range(NCH // 2):
                ps = ps_sc.tile([128, 1024], F32, tag="sc_ps")
                for k in range(2):
                    mch = pr * 2 + k
                    nc.tensor.matmul(
                        ps[:, k * 512:(k + 1) * 512],
                        qkT[br][ko:ko + HD, kc, mch * 128:(mch + 1) * 128],
                        qkT[br][qo:qo + HD, qc, nb * 512:(nb + 1) * 512],
                        start=True, stop=True)
                nc.scalar.activation(
                    eT[:, pr * 2:pr * 2 + 2, :].rearrange("p m q -> p (m q)"),
                    ps[:], Exp)
            return eT

        def emit_avmm(br, h, nb, eT):
            # TRANSPOSED AV: out[n, d] with eT as stationary, v as moving --
            # the moving width is only 65 (64 dims + ones column for the row
            # sums), so this runs at half the moving cost of the d-major
            # form.  Row sums land per-partition: normalization is a single
            # strided reciprocal + one broadcast multiply on DVE, no PE
            # expander needed.
            av = ps_av.tile([128, 4, HD + 1], F32, tag="av_ps")
            for j in range(4):
                for mch in range(NCH):
                    nc.tensor.matmul(
                        av[:, j, :],
                        eT[:, mch, j * 128:(j + 1) * 128],
                        v_aug[br][:, h, mch, :],
                        start=(mch == 0), stop=(mch == NCH - 1))
            slot = rslot(br, h, nb)
            nc.vector.reciprocal(rT[br][:, slot, :], av[:, :, HD])
            rb = rT[br][:, slot, :].rearrange("p (j o) -> p j o", o=1) \
                .broadcast_to([128, 4, HD])
            nc.vector.tensor_tensor(
                out=u_n[br][:, nb * 4:(nb + 1) * 4, h, :],
                in0=av[:, :, 0:HD], in1=rb, op=mybir.AluOpType.mult)
            return av

        def emit_fin(br, h, nb, av):
            # transpose the normalized n-major chunk into the d-major uT the
            # projection consumes (4 PE transposes + one DVE copy); the tp
            # tiles share the av psum ring (identical slot size)
            tp = ps_av.tile([HD, 4, 130], BF16, tag="av_ps")
            for j in range(4):
                nc.tensor.transpose(
                    tp[:, j, 0:128], u_n[br][:, nb * 4 + j, h, :], identb[:])
            uc, uo = divmod(h * HD, 128)
            nc.vector.tensor_copy(
                u_t[br][uo:uo + HD, uc, nb * 512:(nb + 1) * 512]
                .rearrange("p (j q) -> p j q", j=4),
                tp[:, :, 0:128])

        def proj_half(br, half):
            # bias/2 folded into each partial (the pair reduce completes it);
            # stores on DVE; one DMA per slot so the RS has few waits
            u = u_t[br]
            wp = wp_r[br]
            st = pj.tile([128, 2, 4, CW], BF16, tag="pj_st")
            cc = cc_in_d[half] if br == "d" else cc_in_c[half]
            # slot-major so slot 0's store DMA overlaps slot 1's matmuls:
            # the ReduceScatter emission chain shortens by one transfer
            for slot in range(2):
                for nch in range(half * 4, half * 4 + 4):
                    j = nch - half * 4
                    ps = ps_pj.tile([128, CW], F32, tag="pj_ps")
                    last = CW // 128 - 1
                    for ci in range(CW // 128):
                        nc.tensor.matmul(
                            ps[:],
                            u[:, ci, nch * 128:(nch + 1) * 128],
                            wp[:, ci, slot * CW:(slot + 1) * CW],
                            start=(ci == 0), stop=(ci == last))
                    nc.vector.tensor_add(st[:, slot, j, :], ps[:], bias_b[br][:])
                    if j == 1:
                        nc.sync.dma_start(
                            out=cc[slot][0:256].rearrange("(j p) f -> p j f", p=128),
                            in_=st[:, slot, 0:2, :])
                    elif j == 2:
                        nc.sync.dma_start(
                            out=cc[slot][256:384].rearrange("(j p) f -> p j f", p=128),
                            in_=st[:, slot, 2:3, :])
                nc.sync.dma_start(
                    out=cc[slot][384:NH].rearrange("(j p) f -> p j f", p=128),
                    in_=st[:, slot, 3:4, :])
            nc.gpsimd.collective_compute(
                "ReduceScatter", AluOpType.add, replica_groups=GROUPS,
                ins=[cc], outs=[(cc_out_d if br == "d" else cc_out_c)[half]])

        pins = {}

        def pd_pin(k):
            # gpsimd SWDGE queue: this DMA waits on the diff ReduceScatter
            # and must not head-of-line block the SP store queue
            pin = pdl.tile([128, 4, CW], BF16, tag="p_in", bufs=2,
                           name=f"pin_{k}")
            nc.gpsimd.dma_start(
                out=pin[:],
                in_=cc_out_d[k].rearrange("(j p) f -> p j f", p=128))
            pins[k] = pin

        def emit_o2(h, nb, eT):
            # TRANSPOSED o2 = (attn_cond @ p_d) per head: eT stationary,
            # p_d columns moving (64 wide); normalized by the per-partition
            # r_c and stored straight into the natural o_d layout
            ps_o = ps_av.tile([128, 4, HD + 1], F32, tag="av_ps")
            for j in range(4):
                for mch in range(NCH):
                    pin = pins[0] if mch < 4 else pins[1]
                    nc.tensor.matmul(
                        ps_o[:, j, 0:HD],
                        eT[:, mch, j * 128:(j + 1) * 128],
                        pin[:, mch % 4, h * HD:(h + 1) * HD],
                        start=(mch == 0), stop=(mch == NCH - 1))
            rb = rT["c"][:, h * NB + nb, :].rearrange("p (j o) -> p j o", o=1) \
                .broadcast_to([128, 4, HD])
            o2T = o2p.tile([128, 4, HD], BF16, tag="o2T")
            nc.vector.tensor_tensor(out=o2T[:], in0=ps_o[:, :, 0:HD], in1=rb,
                                    op=mybir.AluOpType.mult)
            nc.sync.dma_start(
                out=o_d_nat[nb * 512:(nb + 1) * 512, h * HD:(h + 1) * HD]
                .rearrange("(j p) f -> p j f", p=128),
                in_=o2T[:])

        # ---------------- the stream ----------------
        # task stream with a trailing avmm (lag 1) / fin (lag 2) discipline;
        # fin(h5, nb) triggers that half's projection + ReduceScatter
        avq = []
        finq = []
        hooks = {}

        def advance(scored=None):
            if avq:
                br, h, nb, eT = avq.pop(0)
                finq.append((br, h, nb, emit_avmm(br, h, nb, eT)))
            if len(finq) > 1 or (scored is None and finq):
                br, h, nb, ps_u = finq.pop(0)
                emit_fin(br, h, nb, ps_u)
                if h == HPC - 1:
                    proj_half(br, nb)
                    hk = hooks.pop((br, nb), None)
                    if hk:
                        hk()
            if scored is not None:
                avq.append(scored)

        with tc.tile_pool(name="cond_ld", bufs=1) as cond_ld, \
             tc.tile_pool(name="dld", bufs=1) as dld:
            wqk_cr = cond_ld.tile([128, CCH, 2 * CW], BF16, tag="wqk_c", name="wqk_cr")
            wv_cr = cond_ld.tile([128, CCH, CW], BF16, tag="wv_c", name="wv_cr")
            xT_c = cond_ld.tile([128, CCH, N], BF16, tag="xT_c", name="xT_c")
            wqk_dr = dld.tile([128, CCH, 2 * CW], BF16, tag="wqk_d", name="wqk_dr")
            wv_dr = dld.tile([128, CCH, CW], BF16, tag="wv_d", name="wv_dr")
            xT_d = dld.tile([128, CCH, N], BF16, tag="xT_d", name="xT_d")

            # ---- loads: diff first (its QKV starts the kernel), cond +
            # proj weights + bias stream behind on the same queue ----
            load_qkv("d", wqk_dr, wv_dr, xT_d)
            load_qkv("c", wqk_cr, wv_cr, xT_c)
            for br in ("d", "c"):
                nc.sync.dma_start(out=wp_r[br][:], in_=wp_d if br == "d" else wp_c)
            nc.sync.dma_start(out=bias_b["d"][:], in_=bias_d.to_broadcast([128, CW]))
            nc.sync.dma_start(out=bias_b["c"][:], in_=bias_c.to_broadcast([128, CW]))

            # ---- QKV_d, then the diff attention with the cond QKV tasks
            # interleaved one-per-unit into the exp-bound units' PE slack ----
            for t in qkv_tasks("d", wqk_dr, wv_dr, xT_d):
                t()

            ctasks = qkv_tasks("c", wqk_cr, wv_cr, xT_c)
            ci = 0
            hooks[("d", 1)] = lambda: (pd_pin(0), pd_pin(1))
            for nb in range(NB):
                for h in range(HPC):
                    eT = emit_scores("d", h, nb, dside, "eT_d", 2)
                    advance(("d", h, nb, eT))
                    if nb + h > 0 and ci < len(ctasks):
                        ctasks[ci]()
                        ci += 1
            while ci < len(ctasks):
                ctasks[ci]()
                ci += 1
                advance()

        # load pools closed: the cond eT ring reuses their region.
        # group order c-nb0, d-nb1, c-nb1 packs the four ReduceScatters
        # back to back (d0 early, then c0/d1/c1 as their fins complete)
        with tc.tile_pool(name="eTc", bufs=1) as eTc:
            o2q = []
            for nb in range(NB):
                for h in range(HPC):
                    eT = emit_scores("c", h, nb, eTc, "eT_c", 2 * HPC)
                    o2q.append((h, nb, eT))
                    advance(("c", h, nb, eT))
            while avq or finq:
                advance()

            # ---- drain: all 12 second-attention units overlap the
            # last cond ReduceScatter ----
            nc.gpsimd.dma_start(out=o_c_full[0:NH, :], in_=cc_out_c[0])
            for h, nb, eT in o2q:
                emit_o2(h, nb, eT)
            nc.sync.dma_start(out=o_c_full[NH:NH + 176, :],
                              in_=cc_out_c[1][0:176, :])
            nc.scalar.dma_start(out=o_c_full[NH + 176:NH + 352, :],
                                in_=cc_out_c[1][176:352, :])
            nc.gpsimd.dma_start(out=o_c_full[NH + 352:N, :],
                                in_=cc_out_c[1][352:NH, :])


def _prep_inputs(x, w_qkv_diff, w_qkv_cond, w_proj_diff, b_proj_diff,
                 w_proj_cond, b_proj_cond):
    import ml_dtypes
    bf = ml_dtypes.bfloat16
    in_maps = []
    wqk = {}
    wvv = {}
    for hg in range(2):
        s = slice(hg * CW, (hg + 1) * CW)
        sk = slice(C + hg * CW, C + (hg + 1) * CW)
        sv = slice(2 * C + hg * CW, 2 * C + (hg + 1) * CW)
        for br, w in (("d", w_qkv_diff), ("c", w_qkv_cond)):
            wqk[(br, hg)] = np.ascontiguousarray(
                np.concatenate([w[:, s] * SCALE, w[:, sk]], axis=1)
                .reshape(CCH, 128, 2 * CW).transpose(1, 0, 2).astype(bf))
            wvv[(br, hg)] = np.ascontiguousarray(
                w[:, sv].reshape(CCH, 128, CW).transpose(1, 0, 2).astype(bf))
    wp = {}
    for br, w in (("d", w_proj_diff), ("c", w_proj_cond)):
        for hg in range(2):
            s = slice(hg * CW, (hg + 1) * CW)
            wp[(br, hg)] = np.ascontiguousarray(
                w[s, :].reshape(CW // 128, 128, C).transpose(1, 0, 2).astype(bf))
    for c in range(N_CORES):
        b, hg = divmod(c, 2)
        s = slice(hg * CW, (hg + 1) * CW)
        m = {
            "xt": np.ascontiguousarray(
                x[b].T.reshape(CCH, 128, N0).transpose(1, 0, 2).astype(bf)),
            "wqk_d": wqk[("d", hg)],
            "wqk_c": wqk[("c", hg)],
            "wv_d": wvv[("d", hg)],
            "wv_c": wvv[("c", hg)],
            "wp_d": wp[("d", hg)],
            "wp_c": wp[("c", hg)],
            "bias_d": np.ascontiguousarray((0.5 * b_proj_diff[None, s]).astype(bf)),
            "bias_c": np.ascontiguousarray((0.5 * b_proj_cond[None, s]).astype(bf)),
        }
        in_maps.append(m)
    return in_maps


def kernel(x, w_qkv_diff, w_qkv_cond, w_proj_diff, b_proj_diff,
           w_proj_cond, b_proj_cond):
    x = np.asarray(x)
    w_qkv_diff = np.asarray(w_qkv_diff)
    w_qkv_cond = np.asarray(w_qkv_cond)
    w_proj_diff = np.asarray(w_proj_diff)
    b_proj_diff = np.asarray(b_proj_diff)
    w_proj_cond = np.asarray(w_proj_cond)
    b_proj_cond = np.asarray(b_proj_cond)

    if "nc" not in _CACHE:
        _CACHE["nc"] = _build()
    nc = _CACHE["nc"]
    in_maps = _prep_inputs(x, w_qkv_diff, w_qkv_cond, w_proj_diff,
                           b_proj_diff, w_proj_cond, b_proj_cond)
    res = run_bass_kernel_spmd(nc, in_maps, list(range(N_CORES))).results

    o_d = np.empty((B, N, C), np.float32)
    o_c = np.empty((B, N, C), np.float32)
    for c in range(N_CORES):
        b, hg = divmod(c, 2)
        o_d[b][:, hg * CW:(hg + 1) * CW] = np.asarray(res[c]["o_d_nat"]).astype(np.float32)
        o_c[b][:, hg * CW:(hg + 1) * CW] = np.asarray(res[c]["o_c_cols"]).astype(np.float32)
    return np.concatenate([o_d, o_c], axis=1)
